# revision 1
# baseline (speedup 1.0000x reference)
"""Gated DeltaNet mixer on 8 trn2 NeuronCores.

Sharding: core c -> (batch b = c//2, head-half hh = c%2).  Each core computes
its batch's projections for its 8 heads, runs the chunked gated-delta-rule
scan (C=128, WY form, truncated-Neumann intra-chunk solve), gates, and emits
  pT_c  = ((y * g_norm * gate) @ Wo_half)^T        [1024, 2048]
  ssq_c = sum_d y[t,d]^2 over this half's 512 dims [1, 2048]
Host combines: out[b] = rsqrt((ssq0+ssq1)/1024 + eps)[:,None] * (pT0+pT1).T
(The rmsnorm scalar commutes past the Wo matmul.)

Schedule (engine-balanced, ~352us/core on the CoreSim cost model, 2.8x over
the first working version):
 - x arrives host-transposed, so x^T is a straight DMA (quarter-granular
   so the first projection starts early); projections then stream the PE at
   100% (full P-state).  ALL log-domain work (l2-norm rsqrt + log-sigmoid)
   batches into two Ln activations to avoid act-table reloads (was 65).
   k stays RAW in SBUF: ln|k| folds algebraically into the shared exp bias
   (rows), v1 (columns) and nbgp (ks0 term), so k never needs normalizing.
 - scan: chunk-outer over parity-head PAIRS (even head on partitions 0-63,
   odd on 64-127) so 8 independent recurrences pipeline and the pair shares
   one erow/qg/rv/r/z/yt/s0 op where per-head scalars allow.  Per head the
   e1s|e2s tiles fuse into one [128,256] Exp shifted by -55 so off-mask
   entries stay finite in bf16; a 0/+-e^55 mask constant unshifts and masks
   both heads in one Pool TT.  The log-value row-broadcasts run as hi+lo
   bf16 matmul pairs accumulating the f32 value in PSUM (4x cheaper than
   f32 matmuls).  Neumann solve uses 2 applies (error-neutral vs 4), each
   a bf16 (-W)@Z matmul plus one paired DVE add; the state's bf16 shadow
   copy runs on Pool to keep the cross-chunk chain off DVE.
 - phase E is interleaved per 2-chunk group (gate Silu, zt, ssq, Wo matmul,
   pt DMA SP-issued) to fill scan bubbles and keep the output drain short.
PSUM tags pack multiple per-iteration tiles into single banks (bank-granular
allocator) so rotation depth 2 fits in 8 banks.
"""

import numpy as np
import ml_dtypes
from contextlib import ExitStack

import concourse.bass as bass
import concourse.bacc as bacc_mod
import concourse.tile as tile
from concourse import mybir
from concourse.bass_utils import run_bass_kernel_spmd
from concourse.masks import make_identity

F32 = mybir.dt.float32
BF16 = mybir.dt.bfloat16
AF = mybir.ActivationFunctionType
ALU = mybir.AluOpType

B, S, D = 4, 2048, 1024
H, HD = 16, 64          # global heads
HL = 8                  # heads per core
DL = HL * HD            # 512 dims per core
C = 128                 # chunk length
NCH = S // C            # 16 chunks
NT = S // 128           # 16 time tiles (== chunks)
KD = D // 128           # 8 contraction tiles
NEUMANN = 3             # series terms (4 applies)
BIG = 1e30
SHIFT = 55.0            # exponent shift keeping masked exps finite
E55 = float(np.exp(55.0))

_cache = {}


def _build():
    nc = bacc_mod.Bacc()
    xt_d = nc.dram_tensor("xt", [D, S], BF16, kind="ExternalInput")
    wq_d = nc.dram_tensor("wq", [D, DL], BF16, kind="ExternalInput")
    wk_d = nc.dram_tensor("wk", [D, DL], BF16, kind="ExternalInput")
    wv_d = nc.dram_tensor("wv", [D, DL], BF16, kind="ExternalInput")
    wab_d = nc.dram_tensor("wab", [D, 2 * HL], BF16, kind="ExternalInput")
    wg_d = nc.dram_tensor("wg", [D, DL], BF16, kind="ExternalInput")
    wo_d = nc.dram_tensor("wo", [DL, D], BF16, kind="ExternalInput")
    gn_d = nc.dram_tensor("gn", [DL], F32, kind="ExternalInput")
    pt_d = nc.dram_tensor("pt", [D, S], F32, kind="ExternalOutput")
    ssq_d = nc.dram_tensor("ssq", [1, S], F32, kind="ExternalOutput")

    with ExitStack() as ctx:
        tc = ctx.enter_context(tile.TileContext(nc))
        const = ctx.enter_context(tc.tile_pool(name="const", bufs=1))
        persist = ctx.enter_context(tc.tile_pool(name="persist", bufs=1))

        # ---- constants ----
        ident = const.tile([128, 128], F32)
        make_identity(nc, ident)
        identb = const.tile([128, 128], BF16)
        nc.vector.tensor_copy(identb, ident)
        # LT[p, m] = 1 iff p <= m  (lhsT for inclusive cumsum along positions)
        lt = const.tile([128, 128], F32)
        nc.vector.memset(lt, 1.0)
        nc.gpsimd.affine_select(out=lt, in_=lt, compare_op=ALU.is_ge,
                                fill=0.0, base=0, pattern=[[1, 128]],
                                channel_multiplier=-1)
        # 0/1 masks (bf16) in [sigma(part), t(free)]; strict half pre-negated
        # so wt comes out negated for the Neumann add.
        m01 = const.tile([128, 4, 128], BF16)   # [-strict|incl] x 2 heads
        for hh in (0, 2):
            nc.vector.memset(m01[:, hh, :], -E55)    # sigma < t -> -E55 else 0
            nc.gpsimd.affine_select(out=m01[:, hh, :], in_=m01[:, hh, :],
                                    compare_op=ALU.is_ge, fill=0.0, base=-1,
                                    pattern=[[1, 128]], channel_multiplier=-1)
            nc.vector.memset(m01[:, hh + 1, :], E55)  # sigma <= t -> +E55 else 0
            nc.gpsimd.affine_select(out=m01[:, hh + 1, :], in_=m01[:, hh + 1, :],
                                    compare_op=ALU.is_ge, fill=0.0, base=0,
                                    pattern=[[1, 128]], channel_multiplier=-1)
        ones_col = const.tile([128, 1], BF16)
        nc.vector.memset(ones_col, 1.0)
        gn_sb = const.tile([128, 4], F32)  # g_norm half, col j = dims j*128..
        nc.gpsimd.dma_start(out=gn_sb, in_=gn_d.rearrange("(j p) -> p j", p=128))

        # ---- persistent activations ----
        xtb = persist.tile([128, KD, S], BF16)       # x^T  [d, t]
        knat = persist.tile([128, NT, DL], BF16)      # k (l2-normed) [t, (l e)]
        vnat = persist.tile([128, NT, DL], BF16)     # v [t, (l e)]
        qnat = persist.tile([128, NT, DL], BF16)     # q (l2-normed) [t, (l e)]
        qt = persist.tile([128, 4, S], BF16)         # q^T [(l e), t] (4 row-tiles)
        kt = persist.tile([128, 4, S], BF16)
        yt = persist.tile([128, 4, S], BF16)         # y^T [(l e), t]
        la_src = persist.tile([128, 128], F32)       # log alpha  [pos, (c l)]
        lb_src = persist.tile([128, 128], F32)       # log beta
        beta_a = persist.tile([128, 128], F32)       # beta
        lg_a = persist.tile([128, 128], F32)         # cumsum log alpha (incl)
        nlg_sh = persist.tile([128, 128], F32)       # -lg_a - SHIFT
        elgp = persist.tile([128, 128], F32)         # exp(lg_a + SHIFT)
        v1_b = persist.tile([128, 128], F32)         # (lg_ex + log beta)^T
        lg_b = persist.tile([128, 128], F32)         # lg_a^T
        v1hi = persist.tile([128, 128], BF16)        # bf16 hi/lo split of v1_b
        v1lo = persist.tile([128, 128], BF16)
        lghi = persist.tile([128, 128], BF16)        # bf16 hi/lo split of lg_b
        lglo = persist.tile([128, 128], BF16)
        nbgp = persist.tile([128, 128], F32)         # -beta*exp(lg_ex)

        # =========== phase B: projections q,k,v,ab + x transpose ===========
        with tc.tile_pool(name="wpool", bufs=1) as wpool, \
             tc.tile_pool(name="ppool", bufs=4) as ppool, \
             tc.tile_pool(name="pj_ps", bufs=2, space="PSUM") as pj_ps:
            wq_sb = wpool.tile([128, KD, DL], BF16, tag="wq")
            wk_sb = wpool.tile([128, KD, DL], BF16, tag="wk")
            wv_sb = wpool.tile([128, KD, DL], BF16, tag="wv")
            wab_sb = wpool.tile([128, KD, 2 * HL], BF16, tag="wab")
            nsq = wpool.tile([128, NT, 2 * HL], F32, tag="nsq")  # |q|^2, |k|^2
            en_all = wpool.tile([128, NT, 2 * HL], F32, tag="en")  # exp(-z_ab)
            rn_all = wpool.tile([128, NT, 2 * HL], F32, tag="rn")
            for w_sb, w_d in ((wq_sb, wq_d), (wk_sb, wk_d), (wv_sb, wv_d)):
                nc.gpsimd.dma_start(out=w_sb, in_=w_d.rearrange("(k p) n -> p k n", p=128))
            nc.gpsimd.dma_start(out=wab_sb, in_=wab_d.rearrange("(k p) n -> p k n", p=128))

            # pass 1: projections, raw q/k stash, norms, exp(-z).
            # x arrives host-transposed; xtb is a straight DMA.
            xtv = xt_d.rearrange("(k p) t -> p k t", p=128)
            for qtr in range(4):
                hs = slice(qtr * (S // 4), (qtr + 1) * (S // 4))
                for d in range(KD):
                    nc.sync.dma_start(out=xtb[:, d, hs], in_=xtv[:, d, hs])
            for m in range(NT):
                # projections for this time tile
                ps_q = pj_ps.tile([128, DL], F32, tag="psq", bufs=2)
                ps_k = pj_ps.tile([128, DL], F32, tag="psk", bufs=2)
                ps_v = pj_ps.tile([128, DL], F32, tag="psv", bufs=2)
                ps_ab = pj_ps.tile([128, 2 * HL], F32, tag="psab", bufs=1)
                for d in range(KD):
                    lw = xtb[:, d, m * 128:(m + 1) * 128]
                    st, sp = d == 0, d == KD - 1
                    nc.tensor.matmul(ps_q, lw, wq_sb[:, d, :], start=st, stop=sp)
                    nc.tensor.matmul(ps_k, lw, wk_sb[:, d, :], start=st, stop=sp)
                    nc.tensor.matmul(ps_v, lw, wv_sb[:, d, :], start=st, stop=sp)
                    nc.tensor.matmul(ps_ab, lw, wab_sb[:, d, :], start=st, stop=sp)
                nc.scalar.activation(vnat[:, m, :], ps_v, AF.Copy)
                nc.scalar.activation(qnat[:, m, :], ps_q, AF.Copy)
                nc.scalar.activation(knat[:, m, :], ps_k, AF.Copy)
                nc.scalar.activation(en_all[:, m, :], ps_ab, AF.Exp, scale=-1.0)
                for i, src in enumerate((qnat, knat)):
                    sqb = ppool.tile([128, DL], BF16, tag=f"sq{i}")
                    nc.vector.tensor_tensor(sqb, src[:, m, :], src[:, m, :],
                                            op=ALU.mult)
                    nc.vector.tensor_reduce(
                        nsq[:, m, i * HL:(i + 1) * HL],
                        sqb.rearrange("p (l e) -> p l e", e=HD),
                        axis=mybir.AxisListType.X, op=ALU.add)

            # pass 2: batched logs (exactly two Ln activations in the kernel)
            nlt = wpool.tile([128, NT, 2 * HL], F32, tag="nlt")
            spt = wpool.tile([128, NT, 2 * HL], F32, tag="spt")
            sp1 = ppool.tile([128, NT, 2 * HL], F32, tag="sp1", bufs=1)
            nc.vector.tensor_scalar_add(sp1, en_all, 1.0)   # 1+exp(-z)
            nc.scalar.activation(nlt, nsq, AF.Ln)
            nc.scalar.activation(spt, sp1, AF.Ln)           # softplus(-z)
            nc.scalar.activation(rn_all, nlt, AF.Exp, scale=-0.5)
            lav = la_src.rearrange("p (c l) -> p c l", l=HL)
            lbv = lb_src.rearrange("p (c l) -> p c l", l=HL)
            nc.vector.tensor_scalar_mul(lav, spt[:, :, 0:HL], -1.0)
            nc.vector.tensor_scalar_mul(lbv, spt[:, :, HL:2 * HL], -1.0)
            nc.scalar.activation(beta_a.rearrange("p (c l) -> p c l", l=HL),
                                 spt[:, :, HL:2 * HL], AF.Exp, scale=-1.0)

            # pass 2.5: l2-normalize q (in place) and k (into knat)
            for m in range(NT):
                rnq = rn_all[:, m, 0:HL].unsqueeze(-1).broadcast_to([128, HL, HD])
                qv = qnat[:, m, :].rearrange("p (l e) -> p l e", e=HD)
                eng = nc.gpsimd if m % 2 == 0 else nc.vector
                eng.tensor_tensor(qv, qv, rnq, op=ALU.mult)


            # =========== phase C: log-gamma pipeline ===========
            ps = pj_ps.tile([128, 128], F32, tag="lgps", bufs=1)
            nc.tensor.matmul(ps, lt, la_src, start=True, stop=True)
            nc.scalar.activation(lg_a, ps, AF.Copy)
            nc.vector.tensor_scalar(nlg_sh, lg_a, -1.0, -SHIFT,
                                    op0=ALU.mult, op1=ALU.add)
            # fold ln|k| row-terms into the shared bias (k stays raw in SBUF)
            nltk_v = nlt[:, :, HL:2 * HL]
            rnk_v = rn_all[:, :, HL:2 * HL]
            v3 = lambda t: t.rearrange("p (c l) -> p c l", l=HL)
            nc.vector.scalar_tensor_tensor(v3(nlg_sh), nltk_v, -0.5, v3(nlg_sh),
                                           op0=ALU.mult, op1=ALU.add)
            nc.scalar.activation(elgp, nlg_sh, AF.Exp, scale=-1.0)
            lgex = ppool.tile([128, 128], F32, tag="lgex")
            nc.vector.tensor_sub(lgex, lg_a, la_src)
            egex = ppool.tile([128, 128], F32, tag="egex")
            nc.scalar.activation(egex, lgex, AF.Exp)
            nc.vector.scalar_tensor_tensor(nbgp, egex, -1.0, beta_a,
                                           op0=ALU.mult, op1=ALU.mult)
            nc.vector.tensor_tensor(v3(nbgp), v3(nbgp), rnk_v, op=ALU.mult)
            v1a = ppool.tile([128, 128], F32, tag="v1a")
            nc.vector.tensor_add(v1a, lgex, lb_src)
            nc.vector.scalar_tensor_tensor(v3(v1a), nltk_v, -0.5, v3(v1a),
                                           op0=ALU.mult, op1=ALU.add)
            ps2 = pj_ps.tile([128, 128], F32, tag="lgps", bufs=1)
            nc.tensor.transpose(ps2, v1a, ident)
            nc.scalar.activation(v1_b, ps2, AF.Copy)
            ps3 = pj_ps.tile([128, 128], F32, tag="lgps", bufs=1)
            nc.tensor.transpose(ps3, lg_a, ident)
            nc.scalar.activation(lg_b, ps3, AF.Copy)
            # hi/lo bf16 splits: hi + lo == f32 value to ~1e-3 abs, so the
            # scan's row-broadcast matmuls can run at bf16 rate
            for full, hi, lo in ((v1_b, v1hi, v1lo), (lg_b, lghi, lglo)):
                nc.vector.tensor_copy(hi, full)
                nc.vector.tensor_sub(lo, full, hi)

        # ====== phase D+E: scan with interleaved transposes + output ======
        with tc.tile_pool(name="spool", bufs=6) as sp, \
             tc.tile_pool(name="state", bufs=1) as statep, \
             tc.tile_pool(name="sc_ps", bufs=2, space="PSUM") as scps:
            wg_sb = statep.tile([128, KD, DL], BF16, tag="wg")
            wo_sb = statep.tile([128, 4, D], BF16, tag="wo")
            nc.gpsimd.dma_start(out=wg_sb, in_=wg_d.rearrange("(k p) n -> p k n", p=128))
            nc.gpsimd.dma_start(out=wo_sb, in_=wo_d.rearrange("(j p) n -> p j n", p=128))
            # states: head parity on partitions (odd heads at base 64)
            s0 = statep.tile([128, HL // 2, HD], F32)
            s0b = statep.tile([128, HL // 2, HD], BF16)
            nc.vector.memset(s0, 0.0)
            nc.vector.memset(s0b, 0.0)
            for c in range(NCH):
                # build q^T/k^T column tiles for this chunk
                for srcb, dst in ((qnat[:, c, :], qt), (knat[:, c, :], kt)):
                    for j in range(4):
                        tps = scps.tile([128, 128], BF16, tag="g", bufs=2)
                        nc.tensor.transpose(tps, srcb[:, j * 128:(j + 1) * 128], identb)
                        dd = dst[:, j, c * 128:(c + 1) * 128]
                        nc.scalar.activation(dd, tps, AF.Copy)
                for jp in range(4):        # parity head pair (2jp, 2jp+1)
                    ccols = slice(c * 128, (c + 1) * 128)
                    kthf = kt[:, jp, ccols]
                    qthf = qt[:, jp, ccols]
                    hd_ = []
                    erow2 = sp.tile([128, 128], F32, tag="erow")
                    g12 = scps.tile([128, 4, 128], F32, tag="g", bufs=2)
                    e12 = sp.tile([128, 4, 128], BF16, tag="e12", bufs=4)
                    me = sp.tile([128, 4, 128], F32, tag="me", bufs=4)
                    kk_ps = scps.tile([128, 4, 128], F32, tag="mm1", bufs=1)
                    for h in (0, 1):
                        u = c * HL + 2 * jp + h
                        iub = bass.AP(tensor=identb.tensor,
                                      offset=identb.offset + u,
                                      ap=[identb.ap[0], [0, 128]])
                        # per head: bcast v1[t] then lg[t] (hi+lo bf16 pairs)
                        gh = g12[:, 2 * h:2 * h + 2, :]
                        nc.tensor.matmul(gh[:, 0, :], iub, v1hi, start=True, stop=False)
                        nc.tensor.matmul(gh[:, 0, :], iub, v1lo, start=False, stop=True)
                        nc.tensor.matmul(gh[:, 1, :], iub, lghi, start=True, stop=False)
                        nc.tensor.matmul(gh[:, 1, :], iub, lglo, start=False, stop=True)
                        nc.scalar.activation(e12[:, 2 * h:2 * h + 2, :], gh, AF.Exp,
                                             bias=nlg_sh[:, u:u + 1])
                    nc.gpsimd.tensor_tensor(me, m01, e12, op=ALU.mult)
                    for h in (0, 1):
                        u = c * HL + 2 * jp + h
                        pb = h * 64
                        psl = slice(pb, pb + 64)
                        kth = kt[psl, jp, ccols]
                        qth = qt[psl, jp, ccols]
                        # erow2[h rows] = exp(lg[t]) for this head (exact unshift)
                        nc.gpsimd.tensor_tensor(
                            erow2[psl, :], e12[psl, 2 * h + 1, :],
                            elgp[psl, u:u + 1].broadcast_to([64, 128]), op=ALU.mult)
                        akk = kk_ps[:, 2 * h, :]
                        aqk_ps = kk_ps[:, 2 * h + 1, :]
                        nc.tensor.matmul(akk, kth, kth, start=True, stop=True)
                        wt = sp.tile([128, 128], BF16, tag="wt", bufs=8)
                        nc.vector.tensor_tensor(wt, me[:, 2 * h, :], akk, op=ALU.mult)
                        nc.tensor.matmul(aqk_ps, kth, qth, start=True, stop=True)
                        aqk = sp.tile([128, 128], BF16, tag="aqk", bufs=8)
                        nc.vector.tensor_tensor(aqk, aqk_ps, me[:, 2 * h + 1, :],
                                                op=ALU.mult)
                        hd_.append((u, pb, psl, kth, qth, me, wt, aqk))
                    # paired: Q^T * gamma_t via the stitched erow2
                    qg = sp.tile([128, 128], BF16, tag="qg")
                    nc.gpsimd.tensor_tensor(qg, qthf, erow2, op=ALU.mult)
                    # paired RHS: R = beta*V - (beta*gamma_ex) .* (K @ S0)
                    u0 = c * HL + 2 * jp
                    rv = sp.tile([128, 2, HD], F32, tag="rv")
                    nc.gpsimd.tensor_tensor(
                        rv, vnat[:, c, 2 * jp * HD:(2 * jp + 2) * HD]
                        .rearrange("p (h e) -> p h e", e=HD),
                        beta_a[:, u0:u0 + 2].unsqueeze(-1)
                        .broadcast_to([128, 2, HD]), op=ALU.mult)
                    zbank = scps.tile([128, 8, HD], F32, tag="mm2", bufs=2)
                    osb_ps = scps.tile([128, 5, HD], F32, tag="mm3", bufs=1)
                    r = sp.tile([128, 2, HD], BF16, tag="r")
                    for h, (u, pb, psl, kth, qth, me, wt, aqk) in enumerate(hd_):
                        ks0 = zbank[:, h, :]
                        nc.tensor.matmul(ks0, kth, s0b[psl, jp, :], start=True,
                                         stop=True)
                        nc.vector.scalar_tensor_tensor(r[:, h, :], ks0,
                                                       nbgp[:, u:u + 1], rv[:, h, :],
                                                       op0=ALU.mult, op1=ALU.add)
                    # truncated Neumann: Z <- R + (-W) Z, both heads per step
                    z = r
                    for it in range(NEUMANN - 1):
                        zp = zbank[:, 2 + 2 * it:4 + 2 * it, :]
                        for h, (u, pb, psl, kth, qth, me, wt, aqk) in enumerate(hd_):
                            nc.tensor.matmul(zp[:, h, :], wt, z[:, h, :],
                                             start=True, stop=True)
                        z2 = sp.tile([128, 2, HD], BF16, tag=f"z{it % 2}")
                        nc.vector.tensor_add(z2, r, zp)
                        z = z2
                    # O^T = U^T AqkT + S0^T (gamma Q^T), both heads in one tile
                    ot = osb_ps[:, 0:2, :].rearrange("p a b -> p (a b)")
                    for h, (u, pb, psl, kth, qth, me, wt, aqk) in enumerate(hd_):
                        nc.tensor.matmul(ot[psl, :], z[:, h, :], aqk, start=True,
                                         stop=False, tile_position=(0, pb))
                        nc.tensor.matmul(ot[psl, :], s0b[psl, jp, :], qg[psl, :],
                                         start=False, stop=True,
                                         tile_position=(pb, pb))
                    ytd = yt[:, jp, ccols]
                    nc.scalar.activation(ytd, ot, AF.Copy)
                    # state update; ubar unshifts e2s[127] by E55 (in the mask)
                    snew = osb_ps[:, 2, :]
                    for h, (u, pb, psl, kth, qth, me, wt, aqk) in enumerate(hd_):
                        ubar = sp.tile([128, HD], BF16, tag=f"ub{h}")
                        nc.vector.tensor_scalar_mul(ubar, z[:, h, :],
                                                    me[:, 2 * h + 1, 127:128])
                        nc.tensor.matmul(snew[psl, :],
                                         knat[:, c, (2 * jp + h) * HD:
                                              (2 * jp + h + 1) * HD],
                                         ubar, start=True, stop=True,
                                         tile_position=(0, pb))
                    nc.vector.scalar_tensor_tensor(s0[:, jp, :], s0[:, jp, :],
                                                   erow2[:, 127:128], snew,
                                                   op0=ALU.mult, op1=ALU.add)
                    nc.gpsimd.tensor_copy(s0b[:, jp, :], s0[:, jp, :])
                egroups = []
                if c % 2 == 1:
                    egroups = [((c - 1) * 128, (c + 1) * 128)]
                for (c0, c1) in egroups:
                    cw = c1 - c0
                    cols = slice(c0, c1)
                    ztg = sp.tile([128, 4, 512], BF16, tag="ztg", bufs=2)
                    sqys = []
                    for j in range(4):
                        gps = scps.tile([128, 512], F32, tag="emm", bufs=2)
                        for d in range(KD):
                            nc.tensor.matmul(gps[:, :cw], wg_sb[:, d, j * 128:(j + 1) * 128],
                                             xtb[:, d, cols], start=(d == 0),
                                             stop=(d == KD - 1))
                        gt = sp.tile([128, 512], BF16, tag="gt", bufs=3)
                        nc.scalar.activation(gt[:, :cw], gps[:, :cw], AF.Silu)
                        nc.vector.scalar_tensor_tensor(ztg[:, j, :cw], yt[:, j, cols],
                                                       gn_sb[:, j:j + 1], gt[:, :cw],
                                                       op0=ALU.mult, op1=ALU.mult)
                        sqy = sp.tile([128, 512], BF16, tag=f"sqy{j}", bufs=1)
                        nc.vector.tensor_tensor(sqy[:, :cw], yt[:, j, cols],
                                                yt[:, j, cols], op=ALU.mult)
                        sqys.append(sqy)
                    spt_ = scps.tile([128, 512], F32, tag="emm", bufs=2)
                    sps = spt_[0:1, :cw]
                    for j in range(4):
                        nc.tensor.matmul(sps, ones_col, sqys[j][:, :cw],
                                         start=(j == 0), stop=(j == 3))
                    ssq_g = sp.tile([1, 512], F32, tag="ssqg", bufs=2)
                    nc.vector.tensor_copy(ssq_g[:, :cw], sps)
                    nc.sync.dma_start(out=ssq_d[:, cols], in_=ssq_g[:, :cw])
                    for mo in range(8):
                        ops_ = scps.tile([128, 512], F32, tag="emm", bufs=2)
                        for j in range(4):
                            nc.tensor.matmul(ops_[:, :cw],
                                             wo_sb[:, j, mo * 128:(mo + 1) * 128],
                                             ztg[:, j, :cw], start=(j == 0),
                                             stop=(j == 3))
                        osb = sp.tile([128, 512], F32, tag="osb", bufs=2)
                        if mo % 2 == 0:
                            nc.scalar.activation(osb[:, :cw], ops_[:, :cw], AF.Copy)
                        else:
                            nc.vector.tensor_copy(osb[:, :cw], ops_[:, :cw])
                        nc.sync.dma_start(out=pt_d[mo * 128:(mo + 1) * 128, cols],
                                          in_=osb[:, :cw])
    nc.compile()
    return nc


def kernel(**inputs):
    x = np.ascontiguousarray(np.asarray(inputs["x"], dtype=np.float32))
    Wq = np.asarray(inputs["Wq"], dtype=np.float32)
    Wk = np.asarray(inputs["Wk"], dtype=np.float32)
    Wv = np.asarray(inputs["Wv"], dtype=np.float32)
    Wa = np.asarray(inputs["Wa"], dtype=np.float32)
    Wb = np.asarray(inputs["Wb"], dtype=np.float32)
    Wg = np.asarray(inputs["Wg"], dtype=np.float32)
    Wo = np.asarray(inputs["Wo"], dtype=np.float32)
    gn = np.asarray(inputs["g_norm"], dtype=np.float32)

    if "nc" not in _cache:
        _cache["nc"] = _build()
    nc = _cache["nc"]

    bf = ml_dtypes.bfloat16
    in_maps = []
    for core in range(8):
        b, hh = core // 2, core % 2
        cs, ch = slice(hh * DL, (hh + 1) * DL), slice(hh * HL, (hh + 1) * HL)
        in_maps.append({
            "xt": np.ascontiguousarray(x[b].T.astype(bf)),
            "wq": np.ascontiguousarray(Wq[:, cs].astype(bf)),
            "wk": np.ascontiguousarray(Wk[:, cs].astype(bf)),
            "wv": np.ascontiguousarray(Wv[:, cs].astype(bf)),
            "wab": np.ascontiguousarray(
                np.concatenate([Wa[:, ch], Wb[:, ch]], axis=1).astype(bf)),
            "wg": np.ascontiguousarray(Wg[:, cs].astype(bf)),
            "wo": np.ascontiguousarray(Wo[cs, :].astype(bf)),
            "gn": np.ascontiguousarray(gn[cs]),
        })
    res = run_bass_kernel_spmd(nc, in_maps, core_ids=list(range(8)))
    _cache["last_result"] = res
    out = np.zeros((B, S, D), np.float32)
    for b in range(B):
        r0, r1 = res.results[2 * b], res.results[2 * b + 1]
        p = (r0["pt"] + r1["pt"]).T
        ssq = (r0["ssq"] + r1["ssq"]).reshape(S, 1)
        inv_rms = 1.0 / np.sqrt(ssq / D + 1e-5)
        out[b] = p * inv_rms
    return out



# revision 2
# speedup vs baseline: 1.0120x; 1.0120x over previous
"""Gated DeltaNet mixer on 8 trn2 NeuronCores.

Sharding: core c -> (batch b = c//2, head-half hh = c%2).  Each core computes
its batch's projections for its 8 heads, runs the chunked gated-delta-rule
scan (C=128, WY form, truncated-Neumann intra-chunk solve), gates, and emits
  pT_c  = ((y * g_norm * gate) @ Wo_half)^T        [1024, 2048]
  ssq_c = sum_d y[t,d]^2 over this half's 512 dims [1, 2048]
Host combines: out[b] = rsqrt((ssq0+ssq1)/1024 + eps)[:,None] * (pT0+pT1).T
(The rmsnorm scalar commutes past the Wo matmul.)

Schedule (engine-balanced, ~352us/core on the CoreSim cost model, 2.8x over
the first working version):
 - x arrives host-transposed, so x^T is a straight DMA (quarter-granular
   so the first projection starts early); projections then stream the PE at
   100% (full P-state).  ALL log-domain work (l2-norm rsqrt + log-sigmoid)
   batches into two Ln activations to avoid act-table reloads (was 65).
   k stays RAW in SBUF: ln|k| folds algebraically into the shared exp bias
   (rows), v1 (columns) and nbgp (ks0 term), so k never needs normalizing.
 - scan: chunk-outer over parity-head PAIRS (even head on partitions 0-63,
   odd on 64-127) so 8 independent recurrences pipeline and the pair shares
   one erow/qg/rv/r/z/yt/s0 op where per-head scalars allow.  Per head the
   e1s|e2s tiles fuse into one [128,256] Exp shifted by -55 so off-mask
   entries stay finite in bf16; a 0/+-e^55 mask constant unshifts and masks
   both heads in one Pool TT.  The log-value row-broadcasts run as hi+lo
   bf16 matmul pairs accumulating the f32 value in PSUM (4x cheaper than
   f32 matmuls).  Neumann solve uses 2 applies (error-neutral vs 4), each
   a bf16 (-W)@Z matmul plus one paired DVE add; the state's bf16 shadow
   copy runs on Pool to keep the cross-chunk chain off DVE.
 - phase E is interleaved per 2-chunk group (gate Silu, zt, ssq, Wo matmul,
   pt DMA SP-issued) to fill scan bubbles and keep the output drain short.
PSUM tags pack multiple per-iteration tiles into single banks (bank-granular
allocator) so rotation depth 2 fits in 8 banks.
"""

import numpy as np
import ml_dtypes
from contextlib import ExitStack

import concourse.bass as bass
import concourse.bacc as bacc_mod
import concourse.tile as tile
from concourse import mybir
from concourse.bass_utils import run_bass_kernel_spmd
from concourse.masks import make_identity

F32 = mybir.dt.float32
BF16 = mybir.dt.bfloat16
AF = mybir.ActivationFunctionType
ALU = mybir.AluOpType

B, S, D = 4, 2048, 1024
H, HD = 16, 64          # global heads
HL = 8                  # heads per core
DL = HL * HD            # 512 dims per core
C = 128                 # chunk length
NCH = S // C            # 16 chunks
NT = S // 128           # 16 time tiles (== chunks)
KD = D // 128           # 8 contraction tiles
NEUMANN = 3             # series terms (4 applies)
BIG = 1e30
SHIFT = 55.0            # exponent shift keeping masked exps finite
E55 = float(np.exp(55.0))

_cache = {}


def _build():
    nc = bacc_mod.Bacc()
    xt_d = nc.dram_tensor("xt", [D, S], BF16, kind="ExternalInput")
    wq_d = nc.dram_tensor("wq", [D, DL], BF16, kind="ExternalInput")
    wk_d = nc.dram_tensor("wk", [D, DL], BF16, kind="ExternalInput")
    wv_d = nc.dram_tensor("wv", [D, DL], BF16, kind="ExternalInput")
    wab_d = nc.dram_tensor("wab", [D, 2 * HL], BF16, kind="ExternalInput")
    wg_d = nc.dram_tensor("wg", [D, DL], BF16, kind="ExternalInput")
    wo_d = nc.dram_tensor("wo", [DL, D], BF16, kind="ExternalInput")
    gn_d = nc.dram_tensor("gn", [DL], F32, kind="ExternalInput")
    pt_d = nc.dram_tensor("pt", [D, S], F32, kind="ExternalOutput")
    ssq_d = nc.dram_tensor("ssq", [1, S], F32, kind="ExternalOutput")

    with ExitStack() as ctx:
        tc = ctx.enter_context(tile.TileContext(nc))
        const = ctx.enter_context(tc.tile_pool(name="const", bufs=1))
        persist = ctx.enter_context(tc.tile_pool(name="persist", bufs=1))

        # ---- constants ----
        ident = const.tile([128, 128], F32)
        make_identity(nc, ident)
        identb = const.tile([128, 128], BF16)
        nc.vector.tensor_copy(identb, ident)
        # LT[p, m] = 1 iff p <= m  (lhsT for inclusive cumsum along positions)
        lt = const.tile([128, 128], F32)
        nc.vector.memset(lt, 1.0)
        nc.gpsimd.affine_select(out=lt, in_=lt, compare_op=ALU.is_ge,
                                fill=0.0, base=0, pattern=[[1, 128]],
                                channel_multiplier=-1)
        # 0/1 masks (bf16) in [sigma(part), t(free)]; strict half pre-negated
        # so wt comes out negated for the Neumann add.
        m01 = const.tile([128, 4, 128], BF16)   # [-strict|incl] x 2 heads
        for hh in (0, 2):
            nc.vector.memset(m01[:, hh, :], -E55)    # sigma < t -> -E55 else 0
            nc.gpsimd.affine_select(out=m01[:, hh, :], in_=m01[:, hh, :],
                                    compare_op=ALU.is_ge, fill=0.0, base=-1,
                                    pattern=[[1, 128]], channel_multiplier=-1)
            nc.vector.memset(m01[:, hh + 1, :], E55)  # sigma <= t -> +E55 else 0
            nc.gpsimd.affine_select(out=m01[:, hh + 1, :], in_=m01[:, hh + 1, :],
                                    compare_op=ALU.is_ge, fill=0.0, base=0,
                                    pattern=[[1, 128]], channel_multiplier=-1)
        ones_col = const.tile([128, 1], BF16)
        nc.vector.memset(ones_col, 1.0)
        gn_sb = const.tile([128, 4], F32)  # g_norm half, col j = dims j*128..
        nc.gpsimd.dma_start(out=gn_sb, in_=gn_d.rearrange("(j p) -> p j", p=128))

        # ---- persistent activations ----
        xtb = persist.tile([128, KD, S], BF16)       # x^T  [d, t]
        knat = persist.tile([128, NT, DL], BF16)      # k (l2-normed) [t, (l e)]
        vnat = persist.tile([128, NT, DL], BF16)     # v [t, (l e)]
        qnat = persist.tile([128, NT, DL], BF16)     # q (l2-normed) [t, (l e)]
        qt = persist.tile([128, 4, S], BF16)         # q^T [(l e), t] (4 row-tiles)
        kt = persist.tile([128, 4, S], BF16)
        yt = persist.tile([128, 4, S], BF16)         # y^T [(l e), t]
        la_src = persist.tile([128, 128], F32)       # log alpha  [pos, (c l)]
        lb_src = persist.tile([128, 128], F32)       # log beta
        beta_a = persist.tile([128, 128], F32)       # beta
        lg_a = persist.tile([128, 128], F32)         # cumsum log alpha (incl)
        nlg_sh = persist.tile([128, 128], F32)       # -lg_a - SHIFT
        elgp = persist.tile([128, 128], F32)         # exp(lg_a + SHIFT)
        v1_b = persist.tile([128, 128], F32)         # (lg_ex + log beta)^T
        lg_b = persist.tile([128, 128], F32)         # lg_a^T
        v1hi = persist.tile([128, 128], BF16)        # bf16 hi/lo split of v1_b
        v1lo = persist.tile([128, 128], BF16)
        lghi = persist.tile([128, 128], BF16)        # bf16 hi/lo split of lg_b
        lglo = persist.tile([128, 128], BF16)
        nbgp = persist.tile([128, 128], F32)         # -beta*exp(lg_ex)

        # =========== phase B: projections q,k,v,ab + x transpose ===========
        with tc.tile_pool(name="wpool", bufs=1) as wpool, \
             tc.tile_pool(name="ppool", bufs=4) as ppool, \
             tc.tile_pool(name="pj_ps", bufs=2, space="PSUM") as pj_ps:
            wq_sb = wpool.tile([128, KD, DL], BF16, tag="wq")
            wk_sb = wpool.tile([128, KD, DL], BF16, tag="wk")
            wv_sb = wpool.tile([128, KD, DL], BF16, tag="wv")
            wab_sb = wpool.tile([128, KD, 2 * HL], BF16, tag="wab")
            nsq = wpool.tile([128, NT, 2 * HL], F32, tag="nsq")  # |q|^2, |k|^2
            en_all = wpool.tile([128, NT, 2 * HL], F32, tag="en")  # exp(-z_ab)
            rn_all = wpool.tile([128, NT, 2 * HL], F32, tag="rn")
            for w_sb, w_d in ((wq_sb, wq_d), (wk_sb, wk_d), (wv_sb, wv_d)):
                nc.gpsimd.dma_start(out=w_sb, in_=w_d.rearrange("(k p) n -> p k n", p=128))
            nc.gpsimd.dma_start(out=wab_sb, in_=wab_d.rearrange("(k p) n -> p k n", p=128))

            # pass 1: projections, raw q/k stash, norms, exp(-z).
            # x arrives host-transposed; xtb is a straight DMA.
            xtv = xt_d.rearrange("(k p) t -> p k t", p=128)
            for qtr in range(4):
                hs = slice(qtr * (S // 4), (qtr + 1) * (S // 4))
                for d in range(KD):
                    nc.sync.dma_start(out=xtb[:, d, hs], in_=xtv[:, d, hs])
            for m in range(NT):
                # projections for this time tile
                ps_q = pj_ps.tile([128, DL], F32, tag="psq", bufs=2)
                ps_k = pj_ps.tile([128, DL], F32, tag="psk", bufs=2)
                ps_v = pj_ps.tile([128, DL], F32, tag="psv", bufs=2)
                ps_ab = pj_ps.tile([128, 2 * HL], F32, tag="psab", bufs=1)
                for d in range(KD):
                    lw = xtb[:, d, m * 128:(m + 1) * 128]
                    st, sp = d == 0, d == KD - 1
                    nc.tensor.matmul(ps_q, lw, wq_sb[:, d, :], start=st, stop=sp)
                    nc.tensor.matmul(ps_k, lw, wk_sb[:, d, :], start=st, stop=sp)
                    nc.tensor.matmul(ps_v, lw, wv_sb[:, d, :], start=st, stop=sp)
                    nc.tensor.matmul(ps_ab, lw, wab_sb[:, d, :], start=st, stop=sp)
                nc.scalar.activation(vnat[:, m, :], ps_v, AF.Copy)
                nc.scalar.activation(qnat[:, m, :], ps_q, AF.Copy)
                nc.scalar.activation(knat[:, m, :], ps_k, AF.Copy)
                nc.scalar.activation(en_all[:, m, :], ps_ab, AF.Exp, scale=-1.0)
                for i, src in enumerate((qnat, knat)):
                    sqb = ppool.tile([128, DL], BF16, tag=f"sq{i}")
                    nc.vector.tensor_tensor(sqb, src[:, m, :], src[:, m, :],
                                            op=ALU.mult)
                    nc.vector.tensor_reduce(
                        nsq[:, m, i * HL:(i + 1) * HL],
                        sqb.rearrange("p (l e) -> p l e", e=HD),
                        axis=mybir.AxisListType.X, op=ALU.add)

            # pass 2: batched logs (exactly two Ln activations in the kernel)
            nlt = wpool.tile([128, NT, 2 * HL], F32, tag="nlt")
            spt = wpool.tile([128, NT, 2 * HL], F32, tag="spt")
            sp1 = ppool.tile([128, NT, 2 * HL], F32, tag="sp1", bufs=1)
            nc.vector.tensor_scalar_add(sp1, en_all, 1.0)   # 1+exp(-z)
            nc.scalar.activation(nlt, nsq, AF.Ln)
            nc.scalar.activation(spt, sp1, AF.Ln)           # softplus(-z)
            nc.scalar.activation(rn_all, nlt, AF.Exp, scale=-0.5)
            lav = la_src.rearrange("p (c l) -> p c l", l=HL)
            lbv = lb_src.rearrange("p (c l) -> p c l", l=HL)
            nc.vector.tensor_scalar_mul(lav, spt[:, :, 0:HL], -1.0)
            nc.vector.tensor_scalar_mul(lbv, spt[:, :, HL:2 * HL], -1.0)
            nc.scalar.activation(beta_a.rearrange("p (c l) -> p c l", l=HL),
                                 spt[:, :, HL:2 * HL], AF.Exp, scale=-1.0)

            # pass 2.5: l2-normalize q (in place) and k (into knat)
            for m in range(NT):
                rnq = rn_all[:, m, 0:HL].unsqueeze(-1).broadcast_to([128, HL, HD])
                qv = qnat[:, m, :].rearrange("p (l e) -> p l e", e=HD)
                eng = nc.gpsimd if m % 2 == 0 else nc.vector
                eng.tensor_tensor(qv, qv, rnq, op=ALU.mult)


            # =========== phase C: log-gamma pipeline ===========
            ps = pj_ps.tile([128, 128], F32, tag="lgps", bufs=1)
            nc.tensor.matmul(ps, lt, la_src, start=True, stop=True)
            nc.scalar.activation(lg_a, ps, AF.Copy)
            nc.vector.tensor_scalar(nlg_sh, lg_a, -1.0, -SHIFT,
                                    op0=ALU.mult, op1=ALU.add)
            # fold ln|k| row-terms into the shared bias (k stays raw in SBUF)
            nltk_v = nlt[:, :, HL:2 * HL]
            rnk_v = rn_all[:, :, HL:2 * HL]
            v3 = lambda t: t.rearrange("p (c l) -> p c l", l=HL)
            nc.vector.scalar_tensor_tensor(v3(nlg_sh), nltk_v, -0.5, v3(nlg_sh),
                                           op0=ALU.mult, op1=ALU.add)
            nc.scalar.activation(elgp, nlg_sh, AF.Exp, scale=-1.0)
            lgex = ppool.tile([128, 128], F32, tag="lgex")
            nc.vector.tensor_sub(lgex, lg_a, la_src)
            egex = ppool.tile([128, 128], F32, tag="egex")
            nc.scalar.activation(egex, lgex, AF.Exp)
            nc.vector.scalar_tensor_tensor(nbgp, egex, -1.0, beta_a,
                                           op0=ALU.mult, op1=ALU.mult)
            nc.vector.tensor_tensor(v3(nbgp), v3(nbgp), rnk_v, op=ALU.mult)
            v1a = ppool.tile([128, 128], F32, tag="v1a")
            nc.vector.tensor_add(v1a, lgex, lb_src)
            nc.vector.scalar_tensor_tensor(v3(v1a), nltk_v, -0.5, v3(v1a),
                                           op0=ALU.mult, op1=ALU.add)
            ps2 = pj_ps.tile([128, 128], F32, tag="lgps", bufs=1)
            nc.tensor.transpose(ps2, v1a, ident)
            nc.scalar.activation(v1_b, ps2, AF.Copy)
            ps3 = pj_ps.tile([128, 128], F32, tag="lgps", bufs=1)
            nc.tensor.transpose(ps3, lg_a, ident)
            nc.scalar.activation(lg_b, ps3, AF.Copy)
            # hi/lo bf16 splits: hi + lo == f32 value to ~1e-3 abs, so the
            # scan's row-broadcast matmuls can run at bf16 rate
            for full, hi, lo in ((v1_b, v1hi, v1lo), (lg_b, lghi, lglo)):
                nc.vector.tensor_copy(hi, full)
                nc.vector.tensor_sub(lo, full, hi)

        # ====== phase D+E: scan with interleaved transposes + output ======
        with tc.tile_pool(name="spool", bufs=6) as sp, \
             tc.tile_pool(name="state", bufs=1) as statep, \
             tc.tile_pool(name="sc_ps", bufs=2, space="PSUM") as scps:
            wg_sb = statep.tile([128, KD, DL], BF16, tag="wg")
            wo_sb = statep.tile([128, 4, D], BF16, tag="wo")
            nc.gpsimd.dma_start(out=wg_sb, in_=wg_d.rearrange("(k p) n -> p k n", p=128))
            nc.gpsimd.dma_start(out=wo_sb, in_=wo_d.rearrange("(j p) n -> p j n", p=128))
            # states: head parity on partitions (odd heads at base 64)
            s0 = statep.tile([128, HL // 2, HD], F32)
            s0b = statep.tile([128, HL // 2, HD], BF16)
            nc.vector.memset(s0, 0.0)
            nc.vector.memset(s0b, 0.0)
            for c in range(NCH):
                # build q^T/k^T column tiles for this chunk
                for srcb, dst in ((qnat[:, c, :], qt), (knat[:, c, :], kt)):
                    for j in range(4):
                        tps = scps.tile([128, 128], BF16, tag="g", bufs=2)
                        nc.tensor.transpose(tps, srcb[:, j * 128:(j + 1) * 128], identb)
                        dd = dst[:, j, c * 128:(c + 1) * 128]
                        nc.scalar.activation(dd, tps, AF.Copy)
                for jp in range(4):        # parity head pair (2jp, 2jp+1)
                    ccols = slice(c * 128, (c + 1) * 128)
                    kthf = kt[:, jp, ccols]
                    qthf = qt[:, jp, ccols]
                    hd_ = []
                    erow2 = sp.tile([128, 128], F32, tag="erow")
                    g12 = scps.tile([128, 4, 128], F32, tag="g", bufs=2)
                    e12 = sp.tile([128, 4, 128], BF16, tag="e12", bufs=4)
                    me = sp.tile([128, 4, 128], F32, tag="me", bufs=4)
                    kk_ps = scps.tile([128, 4, 128], F32, tag="mm1", bufs=1)
                    for h in (0, 1):
                        u = c * HL + 2 * jp + h
                        iub = bass.AP(tensor=identb.tensor,
                                      offset=identb.offset + u,
                                      ap=[identb.ap[0], [0, 128]])
                        # per head: bcast v1[t] then lg[t] (hi+lo bf16 pairs)
                        gh = g12[:, 2 * h:2 * h + 2, :]
                        nc.tensor.matmul(gh[:, 0, :], iub, v1hi, start=True, stop=False)
                        nc.tensor.matmul(gh[:, 0, :], iub, v1lo, start=False, stop=True)
                        nc.tensor.matmul(gh[:, 1, :], iub, lghi, start=True, stop=False)
                        nc.tensor.matmul(gh[:, 1, :], iub, lglo, start=False, stop=True)
                        nc.scalar.activation(e12[:, 2 * h:2 * h + 2, :], gh, AF.Exp,
                                             bias=nlg_sh[:, u:u + 1])
                    nc.gpsimd.tensor_tensor(me, m01, e12, op=ALU.mult)
                    for h in (0, 1):
                        u = c * HL + 2 * jp + h
                        pb = h * 64
                        psl = slice(pb, pb + 64)
                        kth = kt[psl, jp, ccols]
                        qth = qt[psl, jp, ccols]
                        # erow2[h rows] = exp(lg[t]) for this head (exact unshift)
                        nc.gpsimd.tensor_tensor(
                            erow2[psl, :], e12[psl, 2 * h + 1, :],
                            elgp[psl, u:u + 1].broadcast_to([64, 128]), op=ALU.mult)
                        akk = kk_ps[:, 2 * h, :]
                        aqk_ps = kk_ps[:, 2 * h + 1, :]
                        nc.tensor.matmul(akk, kth, kth, start=True, stop=True)
                        wt = sp.tile([128, 128], BF16, tag="wt", bufs=8)
                        nc.vector.tensor_tensor(wt, me[:, 2 * h, :], akk, op=ALU.mult)
                        nc.tensor.matmul(aqk_ps, kth, qth, start=True, stop=True)
                        aqk = sp.tile([128, 128], BF16, tag="aqk", bufs=8)
                        nc.vector.tensor_tensor(aqk, aqk_ps, me[:, 2 * h + 1, :],
                                                op=ALU.mult)
                        hd_.append((u, pb, psl, kth, qth, me, wt, aqk))
                    # paired: Q^T * gamma_t via the stitched erow2
                    qg = sp.tile([128, 128], BF16, tag="qg")
                    nc.gpsimd.tensor_tensor(qg, qthf, erow2, op=ALU.mult)
                    # paired RHS: R = beta*V - (beta*gamma_ex) .* (K @ S0)
                    u0 = c * HL + 2 * jp
                    rv = sp.tile([128, 2, HD], F32, tag="rv")
                    nc.gpsimd.tensor_tensor(
                        rv, vnat[:, c, 2 * jp * HD:(2 * jp + 2) * HD]
                        .rearrange("p (h e) -> p h e", e=HD),
                        beta_a[:, u0:u0 + 2].unsqueeze(-1)
                        .broadcast_to([128, 2, HD]), op=ALU.mult)
                    zbank = scps.tile([128, 8, HD], F32, tag="mm2", bufs=2)
                    osb_ps = scps.tile([128, 5, HD], F32, tag="mm3", bufs=1)
                    r = sp.tile([128, 2, HD], BF16, tag="r")
                    for h, (u, pb, psl, kth, qth, me, wt, aqk) in enumerate(hd_):
                        ks0 = zbank[:, h, :]
                        nc.tensor.matmul(ks0, kth, s0b[psl, jp, :], start=True,
                                         stop=True)
                        nc.vector.scalar_tensor_tensor(r[:, h, :], ks0,
                                                       nbgp[:, u:u + 1], rv[:, h, :],
                                                       op0=ALU.mult, op1=ALU.add)
                    # truncated Neumann: Z <- R + (-W) Z, both heads per step
                    z = r
                    for it in range(NEUMANN - 1):
                        zp = zbank[:, 2 + 2 * it:4 + 2 * it, :]
                        for h, (u, pb, psl, kth, qth, me, wt, aqk) in enumerate(hd_):
                            nc.tensor.matmul(zp[:, h, :], wt, z[:, h, :],
                                             start=True, stop=True)
                        z2 = sp.tile([128, 2, HD], BF16, tag=f"z{it % 2}")
                        nc.vector.tensor_add(z2, r, zp)
                        z = z2
                    # O^T = U^T AqkT + S0^T (gamma Q^T), both heads in one tile
                    ot = osb_ps[:, 0:2, :].rearrange("p a b -> p (a b)")
                    for h, (u, pb, psl, kth, qth, me, wt, aqk) in enumerate(hd_):
                        nc.tensor.matmul(ot[psl, :], z[:, h, :], aqk, start=True,
                                         stop=False, tile_position=(0, pb))
                        nc.tensor.matmul(ot[psl, :], s0b[psl, jp, :], qg[psl, :],
                                         start=False, stop=True,
                                         tile_position=(pb, pb))
                    ytd = yt[:, jp, ccols]
                    nc.scalar.activation(ytd, ot, AF.Copy)
                    # state update; ubar unshifts e2s[127] by E55 (in the mask)
                    snew = osb_ps[:, 2, :]
                    for h, (u, pb, psl, kth, qth, me, wt, aqk) in enumerate(hd_):
                        ubar = sp.tile([128, HD], BF16, tag=f"ub{h}")
                        nc.vector.tensor_scalar_mul(ubar, z[:, h, :],
                                                    me[:, 2 * h + 1, 127:128])
                        nc.tensor.matmul(snew[psl, :],
                                         knat[:, c, (2 * jp + h) * HD:
                                              (2 * jp + h + 1) * HD],
                                         ubar, start=True, stop=True,
                                         tile_position=(0, pb))
                    nc.vector.scalar_tensor_tensor(s0[:, jp, :], s0[:, jp, :],
                                                   erow2[:, 127:128], snew,
                                                   op0=ALU.mult, op1=ALU.add)
                    nc.gpsimd.tensor_copy(s0b[:, jp, :], s0[:, jp, :])
                egroups = []
                if c % 2 == 1:
                    egroups = [((c - 1) * 128, (c + 1) * 128)]
                for (c0, c1) in egroups:
                    cw = c1 - c0
                    cols = slice(c0, c1)
                    ztg = sp.tile([128, 4, 512], BF16, tag="ztg", bufs=2)
                    sqys = []
                    for j in range(4):
                        gps = scps.tile([128, 512], F32, tag="emm", bufs=2)
                        for d in range(KD):
                            nc.tensor.matmul(gps[:, :cw], wg_sb[:, d, j * 128:(j + 1) * 128],
                                             xtb[:, d, cols], start=(d == 0),
                                             stop=(d == KD - 1))
                        gt = sp.tile([128, 512], BF16, tag="gt", bufs=3)
                        nc.scalar.activation(gt[:, :cw], gps[:, :cw], AF.Silu)
                        nc.vector.scalar_tensor_tensor(ztg[:, j, :cw], yt[:, j, cols],
                                                       gn_sb[:, j:j + 1], gt[:, :cw],
                                                       op0=ALU.mult, op1=ALU.mult)
                        sqy = sp.tile([128, 512], BF16, tag=f"sqy{j}", bufs=1)
                        nc.vector.tensor_tensor(sqy[:, :cw], yt[:, j, cols],
                                                yt[:, j, cols], op=ALU.mult)
                        sqys.append(sqy)
                    spt_ = scps.tile([128, 512], F32, tag="emm", bufs=2)
                    sps = spt_[0:1, :cw]
                    for j in range(4):
                        nc.tensor.matmul(sps, ones_col, sqys[j][:, :cw],
                                         start=(j == 0), stop=(j == 3))
                    ssq_g = sp.tile([1, 512], F32, tag="ssqg", bufs=2)
                    nc.vector.tensor_copy(ssq_g[:, :cw], sps)
                    nc.sync.dma_start(out=ssq_d[:, cols], in_=ssq_g[:, :cw])
                    for mo in range(8):
                        ops_ = scps.tile([128, 512], F32, tag="emm", bufs=2)
                        for j in range(4):
                            nc.tensor.matmul(ops_[:, :cw],
                                             wo_sb[:, j, mo * 128:(mo + 1) * 128],
                                             ztg[:, j, :cw], start=(j == 0),
                                             stop=(j == 3))
                        osb = sp.tile([128, 512], F32, tag="osb", bufs=2)
                        if mo % 2 == 0:
                            nc.scalar.activation(osb[:, :cw], ops_[:, :cw], AF.Copy)
                        else:
                            nc.vector.tensor_copy(osb[:, :cw], ops_[:, :cw])
                        nc.sync.dma_start(out=pt_d[mo * 128:(mo + 1) * 128, cols],
                                          in_=osb[:, :cw])
        import bass_rust as _bass_rust
        from concourse.hw_specs import get_activation_tables

        def _two_table_loads():
            keep = ("natural_log_exp_and_others", "silu_and_others")
            tables = [(k, (v if k in keep else set()))
                      for k, v in get_activation_tables(nc.m.arch).items()]
            _bass_rust.insert_act_table_loads(nc, tables)

        nc.insert_act_table_loads = _two_table_loads
    nc.compile()
    return nc


def kernel(**inputs):
    x = np.ascontiguousarray(np.asarray(inputs["x"], dtype=np.float32))
    Wq = np.asarray(inputs["Wq"], dtype=np.float32)
    Wk = np.asarray(inputs["Wk"], dtype=np.float32)
    Wv = np.asarray(inputs["Wv"], dtype=np.float32)
    Wa = np.asarray(inputs["Wa"], dtype=np.float32)
    Wb = np.asarray(inputs["Wb"], dtype=np.float32)
    Wg = np.asarray(inputs["Wg"], dtype=np.float32)
    Wo = np.asarray(inputs["Wo"], dtype=np.float32)
    gn = np.asarray(inputs["g_norm"], dtype=np.float32)

    if "nc" not in _cache:
        _cache["nc"] = _build()
    nc = _cache["nc"]

    bf = ml_dtypes.bfloat16
    in_maps = []
    for core in range(8):
        b, hh = core // 2, core % 2
        cs, ch = slice(hh * DL, (hh + 1) * DL), slice(hh * HL, (hh + 1) * HL)
        in_maps.append({
            "xt": np.ascontiguousarray(x[b].T.astype(bf)),
            "wq": np.ascontiguousarray(Wq[:, cs].astype(bf)),
            "wk": np.ascontiguousarray(Wk[:, cs].astype(bf)),
            "wv": np.ascontiguousarray(Wv[:, cs].astype(bf)),
            "wab": np.ascontiguousarray(
                np.concatenate([Wa[:, ch], Wb[:, ch]], axis=1).astype(bf)),
            "wg": np.ascontiguousarray(Wg[:, cs].astype(bf)),
            "wo": np.ascontiguousarray(Wo[cs, :].astype(bf)),
            "gn": np.ascontiguousarray(gn[cs]),
        })
    res = run_bass_kernel_spmd(nc, in_maps, core_ids=list(range(8)))
    _cache["last_result"] = res
    out = np.zeros((B, S, D), np.float32)
    for b in range(B):
        r0, r1 = res.results[2 * b], res.results[2 * b + 1]
        p = (r0["pt"] + r1["pt"]).T
        ssq = (r0["ssq"] + r1["ssq"]).reshape(S, 1)
        inv_rms = 1.0 / np.sqrt(ssq / D + 1e-5)
        out[b] = p * inv_rms
    return out



# revision 3
# speedup vs baseline: 1.0189x; 1.0069x over previous
"""Gated DeltaNet mixer on 8 trn2 NeuronCores.

Sharding: core c -> (batch b = c//2, head-half hh = c%2).  Each core computes
its batch's projections for its 8 heads, runs the chunked gated-delta-rule
scan (C=128, WY form, truncated-Neumann intra-chunk solve), gates, and emits
  pT_c  = ((y * g_norm * gate) @ Wo_half)^T        [1024, 2048]
  ssq_c = sum_d y[t,d]^2 over this half's 512 dims [1, 2048]
Host combines: out[b] = rsqrt((ssq0+ssq1)/1024 + eps)[:,None] * (pT0+pT1).T
(The rmsnorm scalar commutes past the Wo matmul.)

Schedule (engine-balanced, ~352us/core on the CoreSim cost model, 2.8x over
the first working version):
 - x arrives host-transposed, so x^T is a straight DMA (quarter-granular
   so the first projection starts early); projections then stream the PE at
   100% (full P-state).  ALL log-domain work (l2-norm rsqrt + log-sigmoid)
   batches into two Ln activations to avoid act-table reloads (was 65).
   k stays RAW in SBUF: ln|k| folds algebraically into the shared exp bias
   (rows), v1 (columns) and nbgp (ks0 term), so k never needs normalizing.
 - scan: chunk-outer over parity-head PAIRS (even head on partitions 0-63,
   odd on 64-127) so 8 independent recurrences pipeline and the pair shares
   one erow/qg/rv/r/z/yt/s0 op where per-head scalars allow.  Per head the
   e1s|e2s tiles fuse into one [128,256] Exp shifted by -55 so off-mask
   entries stay finite in bf16; a 0/+-e^55 mask constant unshifts and masks
   both heads in one Pool TT.  The log-value row-broadcasts run as hi+lo
   bf16 matmul pairs accumulating the f32 value in PSUM (4x cheaper than
   f32 matmuls).  Neumann solve uses 2 applies (error-neutral vs 4), each
   a bf16 (-W)@Z matmul plus one paired DVE add; the state's bf16 shadow
   copy runs on Pool to keep the cross-chunk chain off DVE.
 - phase E is interleaved per 2-chunk group (gate Silu, zt, ssq, Wo matmul,
   pt DMA SP-issued) to fill scan bubbles and keep the output drain short.
PSUM tags pack multiple per-iteration tiles into single banks (bank-granular
allocator) so rotation depth 2 fits in 8 banks.
"""

import numpy as np
import ml_dtypes
from contextlib import ExitStack

import concourse.bass as bass
import concourse.bacc as bacc_mod
import concourse.tile as tile
from concourse import mybir
from concourse.bass_utils import run_bass_kernel_spmd
from concourse.masks import make_identity

F32 = mybir.dt.float32
BF16 = mybir.dt.bfloat16
AF = mybir.ActivationFunctionType
ALU = mybir.AluOpType

B, S, D = 4, 2048, 1024
H, HD = 16, 64          # global heads
HL = 8                  # heads per core
DL = HL * HD            # 512 dims per core
C = 128                 # chunk length
NCH = S // C            # 16 chunks
NT = S // 128           # 16 time tiles (== chunks)
KD = D // 128           # 8 contraction tiles
NEUMANN = 3             # series terms (4 applies)
BIG = 1e30
SHIFT = 55.0            # exponent shift keeping masked exps finite
E55 = float(np.exp(55.0))

_cache = {}


def _build():
    nc = bacc_mod.Bacc()
    xt_d = nc.dram_tensor("xt", [D, S], BF16, kind="ExternalInput")
    wq_d = nc.dram_tensor("wq", [D, DL], BF16, kind="ExternalInput")
    wk_d = nc.dram_tensor("wk", [D, DL], BF16, kind="ExternalInput")
    wv_d = nc.dram_tensor("wv", [D, DL], BF16, kind="ExternalInput")
    wab_d = nc.dram_tensor("wab", [D, 2 * HL], BF16, kind="ExternalInput")
    wg_d = nc.dram_tensor("wg", [D, DL], BF16, kind="ExternalInput")
    wo_d = nc.dram_tensor("wo", [DL, D], BF16, kind="ExternalInput")
    gn_d = nc.dram_tensor("gn", [DL], F32, kind="ExternalInput")
    pt_d = nc.dram_tensor("pt", [D, S], F32, kind="ExternalOutput")
    ssq_d = nc.dram_tensor("ssq", [1, S], F32, kind="ExternalOutput")

    with ExitStack() as ctx:
        tc = ctx.enter_context(tile.TileContext(nc))
        const = ctx.enter_context(tc.tile_pool(name="const", bufs=1))
        persist = ctx.enter_context(tc.tile_pool(name="persist", bufs=1))

        # ---- constants ----
        ident = const.tile([128, 128], F32)
        make_identity(nc, ident)
        identb = const.tile([128, 128], BF16)
        nc.vector.tensor_copy(identb, ident)
        # LT[p, m] = 1 iff p <= m  (lhsT for inclusive cumsum along positions)
        lt = const.tile([128, 128], F32)
        nc.vector.memset(lt, 1.0)
        nc.gpsimd.affine_select(out=lt, in_=lt, compare_op=ALU.is_ge,
                                fill=0.0, base=0, pattern=[[1, 128]],
                                channel_multiplier=-1)
        # 0/1 masks (bf16) in [sigma(part), t(free)]; strict half pre-negated
        # so wt comes out negated for the Neumann add.
        m01 = const.tile([128, 4, 128], BF16)   # [-strict|incl] x 2 heads
        for hh in (0, 2):
            nc.vector.memset(m01[:, hh, :], -E55)    # sigma < t -> -E55 else 0
            nc.gpsimd.affine_select(out=m01[:, hh, :], in_=m01[:, hh, :],
                                    compare_op=ALU.is_ge, fill=0.0, base=-1,
                                    pattern=[[1, 128]], channel_multiplier=-1)
            nc.vector.memset(m01[:, hh + 1, :], E55)  # sigma <= t -> +E55 else 0
            nc.gpsimd.affine_select(out=m01[:, hh + 1, :], in_=m01[:, hh + 1, :],
                                    compare_op=ALU.is_ge, fill=0.0, base=0,
                                    pattern=[[1, 128]], channel_multiplier=-1)
        ones_col = const.tile([128, 1], BF16)
        nc.vector.memset(ones_col, 1.0)
        gn_sb = const.tile([128, 4], F32)  # g_norm half, col j = dims j*128..
        nc.gpsimd.dma_start(out=gn_sb, in_=gn_d.rearrange("(j p) -> p j", p=128))

        # ---- persistent activations ----
        xtb = persist.tile([128, KD, S], BF16)       # x^T  [d, t]
        knat = persist.tile([128, NT, DL], BF16)      # k (l2-normed) [t, (l e)]
        vnat = persist.tile([128, NT, DL], BF16)     # v [t, (l e)]
        qnat = persist.tile([128, NT, DL], BF16)     # q (l2-normed) [t, (l e)]
        qt = persist.tile([128, 4, S], BF16)         # q^T [(l e), t] (4 row-tiles)
        kt = persist.tile([128, 4, S], BF16)
        yt = persist.tile([128, 4, S], BF16)         # y^T [(l e), t]
        la_src = persist.tile([128, 128], F32)       # log alpha  [pos, (c l)]
        lb_src = persist.tile([128, 128], F32)       # log beta
        beta_a = persist.tile([128, 128], F32)       # beta
        lg_a = persist.tile([128, 128], F32)         # cumsum log alpha (incl)
        nlg_sh = persist.tile([128, 128], F32)       # -lg_a - SHIFT
        elgp = persist.tile([128, 128], F32)         # exp(lg_a + SHIFT)
        v1_b = persist.tile([128, 128], F32)         # (lg_ex + log beta)^T
        lg_b = persist.tile([128, 128], F32)         # lg_a^T
        v1hi = persist.tile([128, 128], BF16)        # bf16 hi/lo split of v1_b
        v1lo = persist.tile([128, 128], BF16)
        lghi = persist.tile([128, 128], BF16)        # bf16 hi/lo split of lg_b
        lglo = persist.tile([128, 128], BF16)
        nbgp = persist.tile([128, 128], F32)         # -beta*exp(lg_ex)

        # =========== phase B: projections q,k,v,ab + x transpose ===========
        with tc.tile_pool(name="wpool", bufs=1) as wpool, \
             tc.tile_pool(name="ppool", bufs=4) as ppool, \
             tc.tile_pool(name="pj_ps", bufs=2, space="PSUM") as pj_ps:
            wq_sb = wpool.tile([128, KD, DL], BF16, tag="wq")
            wk_sb = wpool.tile([128, KD, DL], BF16, tag="wk")
            wv_sb = wpool.tile([128, KD, DL], BF16, tag="wv")
            wab_sb = wpool.tile([128, KD, 2 * HL], BF16, tag="wab")
            nsq = wpool.tile([128, NT, 2 * HL], F32, tag="nsq")  # |q|^2, |k|^2
            en_all = wpool.tile([128, NT, 2 * HL], F32, tag="en")  # exp(-z_ab)
            rn_all = wpool.tile([128, NT, 2 * HL], F32, tag="rn")
            for w_sb, w_d in ((wq_sb, wq_d), (wk_sb, wk_d), (wv_sb, wv_d)):
                nc.gpsimd.dma_start(out=w_sb, in_=w_d.rearrange("(k p) n -> p k n", p=128))
            nc.gpsimd.dma_start(out=wab_sb, in_=wab_d.rearrange("(k p) n -> p k n", p=128))

            # pass 1: projections, raw q/k stash, norms, exp(-z).
            # x arrives host-transposed; xtb is a straight DMA.
            xtv = xt_d.rearrange("(k p) t -> p k t", p=128)
            for qtr in range(4):
                hs = slice(qtr * (S // 4), (qtr + 1) * (S // 4))
                for d in range(KD):
                    nc.sync.dma_start(out=xtb[:, d, hs], in_=xtv[:, d, hs])
            for m in range(NT):
                # projections for this time tile
                ps_q = pj_ps.tile([128, DL], F32, tag="psq", bufs=2)
                ps_k = pj_ps.tile([128, DL], F32, tag="psk", bufs=2)
                ps_v = pj_ps.tile([128, DL], F32, tag="psv", bufs=2)
                ps_ab = pj_ps.tile([128, 2 * HL], F32, tag="psab", bufs=1)
                for d in range(KD):
                    lw = xtb[:, d, m * 128:(m + 1) * 128]
                    st, sp = d == 0, d == KD - 1
                    nc.tensor.matmul(ps_q, lw, wq_sb[:, d, :], start=st, stop=sp)
                    nc.tensor.matmul(ps_k, lw, wk_sb[:, d, :], start=st, stop=sp)
                    nc.tensor.matmul(ps_v, lw, wv_sb[:, d, :], start=st, stop=sp)
                    nc.tensor.matmul(ps_ab, lw, wab_sb[:, d, :], start=st, stop=sp)
                nc.scalar.activation(vnat[:, m, :], ps_v, AF.Copy)
                nc.scalar.activation(qnat[:, m, :], ps_q, AF.Copy)
                nc.scalar.activation(knat[:, m, :], ps_k, AF.Copy)
                nc.scalar.activation(en_all[:, m, :], ps_ab, AF.Exp, scale=-1.0)
                for i, src in enumerate((qnat, knat)):
                    sqb = ppool.tile([128, DL], BF16, tag=f"sq{i}")
                    nc.vector.tensor_tensor(sqb, src[:, m, :], src[:, m, :],
                                            op=ALU.mult)
                    nc.vector.tensor_reduce(
                        nsq[:, m, i * HL:(i + 1) * HL],
                        sqb.rearrange("p (l e) -> p l e", e=HD),
                        axis=mybir.AxisListType.X, op=ALU.add)

            # pass 2+C: log pipeline emitted per sequence-half so half 0
            # completes while phase B still streams the second half
            nlt = wpool.tile([128, NT, 2 * HL], F32, tag="nlt")
            spt = wpool.tile([128, NT, 2 * HL], F32, tag="spt")
            sp1 = ppool.tile([128, NT, 2 * HL], F32, tag="sp1", bufs=1)
            v3 = lambda t: t.rearrange("p (c l) -> p c l", l=HL)

            def logc_half(hh):
                ms = slice(hh * NT // 2, (hh + 1) * NT // 2)
                cols = slice(hh * 64, (hh + 1) * 64)
                rows = slice(hh * 64, (hh + 1) * 64)
                nc.vector.tensor_scalar_add(sp1[:, ms, :], en_all[:, ms, :],
                                            1.0)
                nc.scalar.activation(nlt[:, ms, :], nsq[:, ms, :], AF.Ln)
                nc.scalar.activation(spt[:, ms, :], sp1[:, ms, :], AF.Ln)
                nc.scalar.activation(rn_all[:, ms, :], nlt[:, ms, :], AF.Exp,
                                     scale=-0.5)
                lav = la_src.rearrange("p (c l) -> p c l", l=HL)
                lbv = lb_src.rearrange("p (c l) -> p c l", l=HL)
                nc.vector.tensor_scalar_mul(lav[:, ms, :], spt[:, ms, 0:HL],
                                            -1.0)
                nc.vector.tensor_scalar_mul(lbv[:, ms, :],
                                            spt[:, ms, HL:2 * HL], -1.0)
                nc.scalar.activation(
                    beta_a.rearrange("p (c l) -> p c l", l=HL)[:, ms, :],
                    spt[:, ms, HL:2 * HL], AF.Exp, scale=-1.0)
                # l2-normalize q for this half (k stays raw; ln|k| folds)
                for m in range(hh * NT // 2, (hh + 1) * NT // 2):
                    rnq = rn_all[:, m, 0:HL].unsqueeze(-1) \
                        .broadcast_to([128, HL, HD])
                    qv = qnat[:, m, :].rearrange("p (l e) -> p l e", e=HD)
                    eng = nc.gpsimd if m % 2 == 0 else nc.vector
                    eng.tensor_tensor(qv, qv, rnq, op=ALU.mult)
                # log-gamma pipeline, column-sliced
                ps = pj_ps.tile([128, 128], F32, tag="lgt", bufs=1,
                                name=f"lgps{hh}")
                nc.tensor.matmul(ps[:, 0:64], lt, la_src[:, cols], start=True,
                                 stop=True)
                nc.scalar.activation(lg_a[:, cols], ps[:, 0:64], AF.Copy)
                nc.vector.tensor_scalar(nlg_sh[:, cols], lg_a[:, cols], -1.0,
                                        -SHIFT, op0=ALU.mult, op1=ALU.add)
                nltk_v = nlt[:, ms, HL:2 * HL]
                rnk_v = rn_all[:, ms, HL:2 * HL]
                nc.vector.scalar_tensor_tensor(v3(nlg_sh)[:, ms, :], nltk_v,
                                               -0.5, v3(nlg_sh)[:, ms, :],
                                               op0=ALU.mult, op1=ALU.add)
                nc.scalar.activation(elgp[:, cols], nlg_sh[:, cols], AF.Exp,
                                     scale=-1.0)
                lgex = ppool.tile([128, 64], F32, tag="lgex",
                                  name=f"lgex{hh}")
                nc.vector.tensor_sub(lgex, lg_a[:, cols], la_src[:, cols])
                egex = ppool.tile([128, 64], F32, tag="egex",
                                  name=f"egex{hh}")
                nc.scalar.activation(egex, lgex, AF.Exp)
                nc.vector.scalar_tensor_tensor(nbgp[:, cols], egex, -1.0,
                                               beta_a[:, cols],
                                               op0=ALU.mult, op1=ALU.mult)
                nc.vector.tensor_tensor(v3(nbgp)[:, ms, :], v3(nbgp)[:, ms, :],
                                        rnk_v, op=ALU.mult)
                v1a = ppool.tile([128, 64], F32, tag="v1a", name=f"v1a{hh}")
                nc.vector.tensor_add(v1a, lgex, lb_src[:, cols])
                nc.vector.scalar_tensor_tensor(
                    v3(v1a).rearrange("p c l -> p (c l)")
                    .rearrange("p (c l) -> p c l", l=HL), nltk_v, -0.5,
                    v1a.rearrange("p (c l) -> p c l", l=HL),
                    op0=ALU.mult, op1=ALU.add)
                ps2 = pj_ps.tile([128, 128], F32, tag="lgt", bufs=1,
                                 name=f"lgt2{hh}")
                nc.tensor.matmul(ps2[rows, :], v1a, ident, start=True,
                                 stop=True, tile_position=(0, 64 * hh))
                nc.scalar.activation(v1_b[rows, :], ps2[rows, :], AF.Copy)
                ps3 = pj_ps.tile([128, 128], F32, tag="lgt", bufs=1,
                                 name=f"lgt3{hh}")
                nc.tensor.matmul(ps3[rows, :], lg_a[:, cols], ident,
                                 start=True, stop=True,
                                 tile_position=(0, 64 * hh))
                nc.scalar.activation(lg_b[rows, :], ps3[rows, :], AF.Copy)
                for full, hi, lo in ((v1_b, v1hi, v1lo), (lg_b, lghi, lglo)):
                    nc.vector.tensor_copy(hi[rows, :], full[rows, :])
                    nc.vector.tensor_sub(lo[rows, :], full[rows, :],
                                         hi[rows, :])

            logc_half(0)
            logc_half(1)

        # ====== phase D+E: scan with interleaved transposes + output ======
        with tc.tile_pool(name="spool", bufs=6) as sp, \
             tc.tile_pool(name="state", bufs=1) as statep, \
             tc.tile_pool(name="sc_ps", bufs=2, space="PSUM") as scps:
            wg_sb = statep.tile([128, KD, DL], BF16, tag="wg")
            wo_sb = statep.tile([128, 4, D], BF16, tag="wo")
            nc.gpsimd.dma_start(out=wg_sb, in_=wg_d.rearrange("(k p) n -> p k n", p=128))
            nc.gpsimd.dma_start(out=wo_sb, in_=wo_d.rearrange("(j p) n -> p j n", p=128))
            # states: head parity on partitions (odd heads at base 64)
            s0 = statep.tile([128, HL // 2, HD], F32)
            s0b = statep.tile([128, HL // 2, HD], BF16)
            nc.vector.memset(s0, 0.0)
            nc.vector.memset(s0b, 0.0)
            for c in range(NCH):
                # build q^T/k^T column tiles for this chunk
                for srcb, dst in ((qnat[:, c, :], qt), (knat[:, c, :], kt)):
                    for j in range(4):
                        tps = scps.tile([128, 128], BF16, tag="g", bufs=2)
                        nc.tensor.transpose(tps, srcb[:, j * 128:(j + 1) * 128], identb)
                        dd = dst[:, j, c * 128:(c + 1) * 128]
                        nc.scalar.activation(dd, tps, AF.Copy)
                for jp in range(4):        # parity head pair (2jp, 2jp+1)
                    ccols = slice(c * 128, (c + 1) * 128)
                    kthf = kt[:, jp, ccols]
                    qthf = qt[:, jp, ccols]
                    hd_ = []
                    erow2 = sp.tile([128, 128], F32, tag="erow")
                    g12 = scps.tile([128, 4, 128], F32, tag="g", bufs=2)
                    e12 = sp.tile([128, 4, 128], BF16, tag="e12", bufs=4)
                    me = sp.tile([128, 4, 128], F32, tag="me", bufs=4)
                    kk_ps = scps.tile([128, 4, 128], F32, tag="mm1", bufs=1)
                    for h in (0, 1):
                        u = c * HL + 2 * jp + h
                        iub = bass.AP(tensor=identb.tensor,
                                      offset=identb.offset + u,
                                      ap=[identb.ap[0], [0, 128]])
                        # per head: bcast v1[t] then lg[t] (hi+lo bf16 pairs)
                        gh = g12[:, 2 * h:2 * h + 2, :]
                        nc.tensor.matmul(gh[:, 0, :], iub, v1hi, start=True, stop=False)
                        nc.tensor.matmul(gh[:, 0, :], iub, v1lo, start=False, stop=True)
                        nc.tensor.matmul(gh[:, 1, :], iub, lghi, start=True, stop=False)
                        nc.tensor.matmul(gh[:, 1, :], iub, lglo, start=False, stop=True)
                        nc.scalar.activation(e12[:, 2 * h:2 * h + 2, :], gh, AF.Exp,
                                             bias=nlg_sh[:, u:u + 1])
                    nc.gpsimd.tensor_tensor(me, m01, e12, op=ALU.mult)
                    for h in (0, 1):
                        u = c * HL + 2 * jp + h
                        pb = h * 64
                        psl = slice(pb, pb + 64)
                        kth = kt[psl, jp, ccols]
                        qth = qt[psl, jp, ccols]
                        # erow2[h rows] = exp(lg[t]) for this head (exact unshift)
                        nc.gpsimd.tensor_tensor(
                            erow2[psl, :], e12[psl, 2 * h + 1, :],
                            elgp[psl, u:u + 1].broadcast_to([64, 128]), op=ALU.mult)
                        akk = kk_ps[:, 2 * h, :]
                        aqk_ps = kk_ps[:, 2 * h + 1, :]
                        nc.tensor.matmul(akk, kth, kth, start=True, stop=True)
                        wt = sp.tile([128, 128], BF16, tag="wt", bufs=8)
                        nc.vector.tensor_tensor(wt, me[:, 2 * h, :], akk, op=ALU.mult)
                        nc.tensor.matmul(aqk_ps, kth, qth, start=True, stop=True)
                        aqk = sp.tile([128, 128], BF16, tag="aqk", bufs=8)
                        nc.vector.tensor_tensor(aqk, aqk_ps, me[:, 2 * h + 1, :],
                                                op=ALU.mult)
                        hd_.append((u, pb, psl, kth, qth, me, wt, aqk))
                    # paired: Q^T * gamma_t via the stitched erow2
                    qg = sp.tile([128, 128], BF16, tag="qg")
                    nc.gpsimd.tensor_tensor(qg, qthf, erow2, op=ALU.mult)
                    # paired RHS: R = beta*V - (beta*gamma_ex) .* (K @ S0)
                    u0 = c * HL + 2 * jp
                    rv = sp.tile([128, 2, HD], F32, tag="rv")
                    nc.gpsimd.tensor_tensor(
                        rv, vnat[:, c, 2 * jp * HD:(2 * jp + 2) * HD]
                        .rearrange("p (h e) -> p h e", e=HD),
                        beta_a[:, u0:u0 + 2].unsqueeze(-1)
                        .broadcast_to([128, 2, HD]), op=ALU.mult)
                    zbank = scps.tile([128, 8, HD], F32, tag="mm2", bufs=2)
                    osb_ps = scps.tile([128, 5, HD], F32, tag="mm3", bufs=1)
                    r = sp.tile([128, 2, HD], BF16, tag="r")
                    for h, (u, pb, psl, kth, qth, me, wt, aqk) in enumerate(hd_):
                        ks0 = zbank[:, h, :]
                        nc.tensor.matmul(ks0, kth, s0b[psl, jp, :], start=True,
                                         stop=True)
                        nc.vector.scalar_tensor_tensor(r[:, h, :], ks0,
                                                       nbgp[:, u:u + 1], rv[:, h, :],
                                                       op0=ALU.mult, op1=ALU.add)
                    # truncated Neumann: Z <- R + (-W) Z, both heads per step
                    z = r
                    for it in range(NEUMANN - 1):
                        zp = zbank[:, 2 + 2 * it:4 + 2 * it, :]
                        for h, (u, pb, psl, kth, qth, me, wt, aqk) in enumerate(hd_):
                            nc.tensor.matmul(zp[:, h, :], wt, z[:, h, :],
                                             start=True, stop=True)
                        z2 = sp.tile([128, 2, HD], BF16, tag=f"z{it % 2}")
                        nc.vector.tensor_add(z2, r, zp)
                        z = z2
                    # O^T = U^T AqkT + S0^T (gamma Q^T), both heads in one tile
                    ot = osb_ps[:, 0:2, :].rearrange("p a b -> p (a b)")
                    for h, (u, pb, psl, kth, qth, me, wt, aqk) in enumerate(hd_):
                        nc.tensor.matmul(ot[psl, :], z[:, h, :], aqk, start=True,
                                         stop=False, tile_position=(0, pb))
                        nc.tensor.matmul(ot[psl, :], s0b[psl, jp, :], qg[psl, :],
                                         start=False, stop=True,
                                         tile_position=(pb, pb))
                    ytd = yt[:, jp, ccols]
                    nc.scalar.activation(ytd, ot, AF.Copy)
                    # state update; ubar unshifts e2s[127] by E55 (in the mask)
                    snew = osb_ps[:, 2, :]
                    for h, (u, pb, psl, kth, qth, me, wt, aqk) in enumerate(hd_):
                        ubar = sp.tile([128, HD], BF16, tag=f"ub{h}")
                        nc.vector.tensor_scalar_mul(ubar, z[:, h, :],
                                                    me[:, 2 * h + 1, 127:128])
                        nc.tensor.matmul(snew[psl, :],
                                         knat[:, c, (2 * jp + h) * HD:
                                              (2 * jp + h + 1) * HD],
                                         ubar, start=True, stop=True,
                                         tile_position=(0, pb))
                    nc.vector.scalar_tensor_tensor(s0[:, jp, :], s0[:, jp, :],
                                                   erow2[:, 127:128], snew,
                                                   op0=ALU.mult, op1=ALU.add)
                    nc.gpsimd.tensor_copy(s0b[:, jp, :], s0[:, jp, :])
                egroups = []
                if c % 2 == 1:
                    egroups = [((c - 1) * 128, (c + 1) * 128)]
                for (c0, c1) in egroups:
                    cw = c1 - c0
                    cols = slice(c0, c1)
                    ztg = sp.tile([128, 4, 512], BF16, tag="ztg", bufs=2)
                    sqys = []
                    for j in range(4):
                        gps = scps.tile([128, 512], F32, tag="emm", bufs=2)
                        for d in range(KD):
                            nc.tensor.matmul(gps[:, :cw], wg_sb[:, d, j * 128:(j + 1) * 128],
                                             xtb[:, d, cols], start=(d == 0),
                                             stop=(d == KD - 1))
                        gt = sp.tile([128, 512], BF16, tag="gt", bufs=3)
                        nc.scalar.activation(gt[:, :cw], gps[:, :cw], AF.Silu)
                        nc.vector.scalar_tensor_tensor(ztg[:, j, :cw], yt[:, j, cols],
                                                       gn_sb[:, j:j + 1], gt[:, :cw],
                                                       op0=ALU.mult, op1=ALU.mult)
                        sqy = sp.tile([128, 512], BF16, tag=f"sqy{j}", bufs=1)
                        nc.vector.tensor_tensor(sqy[:, :cw], yt[:, j, cols],
                                                yt[:, j, cols], op=ALU.mult)
                        sqys.append(sqy)
                    spt_ = scps.tile([128, 512], F32, tag="emm", bufs=2)
                    sps = spt_[0:1, :cw]
                    for j in range(4):
                        nc.tensor.matmul(sps, ones_col, sqys[j][:, :cw],
                                         start=(j == 0), stop=(j == 3))
                    ssq_g = sp.tile([1, 512], F32, tag="ssqg", bufs=2)
                    nc.vector.tensor_copy(ssq_g[:, :cw], sps)
                    nc.sync.dma_start(out=ssq_d[:, cols], in_=ssq_g[:, :cw])
                    for mo in range(8):
                        ops_ = scps.tile([128, 512], F32, tag="emm", bufs=2)
                        for j in range(4):
                            nc.tensor.matmul(ops_[:, :cw],
                                             wo_sb[:, j, mo * 128:(mo + 1) * 128],
                                             ztg[:, j, :cw], start=(j == 0),
                                             stop=(j == 3))
                        osb = sp.tile([128, 512], F32, tag="osb", bufs=2)
                        if mo % 2 == 0:
                            nc.scalar.activation(osb[:, :cw], ops_[:, :cw], AF.Copy)
                        else:
                            nc.vector.tensor_copy(osb[:, :cw], ops_[:, :cw])
                        nc.sync.dma_start(out=pt_d[mo * 128:(mo + 1) * 128, cols],
                                          in_=osb[:, :cw])
        import bass_rust as _bass_rust
        from concourse.hw_specs import get_activation_tables

        def _two_table_loads():
            keep = ("natural_log_exp_and_others", "silu_and_others")
            tables = [(k, (v if k in keep else set()))
                      for k, v in get_activation_tables(nc.m.arch).items()]
            _bass_rust.insert_act_table_loads(nc, tables)

        nc.insert_act_table_loads = _two_table_loads
    nc.compile()
    return nc


def kernel(**inputs):
    x = np.ascontiguousarray(np.asarray(inputs["x"], dtype=np.float32))
    Wq = np.asarray(inputs["Wq"], dtype=np.float32)
    Wk = np.asarray(inputs["Wk"], dtype=np.float32)
    Wv = np.asarray(inputs["Wv"], dtype=np.float32)
    Wa = np.asarray(inputs["Wa"], dtype=np.float32)
    Wb = np.asarray(inputs["Wb"], dtype=np.float32)
    Wg = np.asarray(inputs["Wg"], dtype=np.float32)
    Wo = np.asarray(inputs["Wo"], dtype=np.float32)
    gn = np.asarray(inputs["g_norm"], dtype=np.float32)

    if "nc" not in _cache:
        _cache["nc"] = _build()
    nc = _cache["nc"]

    bf = ml_dtypes.bfloat16
    in_maps = []
    for core in range(8):
        b, hh = core // 2, core % 2
        cs, ch = slice(hh * DL, (hh + 1) * DL), slice(hh * HL, (hh + 1) * HL)
        in_maps.append({
            "xt": np.ascontiguousarray(x[b].T.astype(bf)),
            "wq": np.ascontiguousarray(Wq[:, cs].astype(bf)),
            "wk": np.ascontiguousarray(Wk[:, cs].astype(bf)),
            "wv": np.ascontiguousarray(Wv[:, cs].astype(bf)),
            "wab": np.ascontiguousarray(
                np.concatenate([Wa[:, ch], Wb[:, ch]], axis=1).astype(bf)),
            "wg": np.ascontiguousarray(Wg[:, cs].astype(bf)),
            "wo": np.ascontiguousarray(Wo[cs, :].astype(bf)),
            "gn": np.ascontiguousarray(gn[cs]),
        })
    res = run_bass_kernel_spmd(nc, in_maps, core_ids=list(range(8)))
    _cache["last_result"] = res
    out = np.zeros((B, S, D), np.float32)
    for b in range(B):
        r0, r1 = res.results[2 * b], res.results[2 * b + 1]
        p = (r0["pt"] + r1["pt"]).T
        ssq = (r0["ssq"] + r1["ssq"]).reshape(S, 1)
        inv_rms = 1.0 / np.sqrt(ssq / D + 1e-5)
        out[b] = p * inv_rms
    return out



# revision 4
# speedup vs baseline: 1.0423x; 1.0229x over previous
"""Gated DeltaNet mixer on 8 trn2 NeuronCores.

Sharding: core c -> (batch b = c//2, head-half hh = c%2).  Each core computes
its batch's projections for its 8 heads, runs the chunked gated-delta-rule
scan (C=128, WY form, truncated-Neumann intra-chunk solve), gates, and emits
  pT_c  = ((y * g_norm * gate) @ Wo_half)^T        [1024, 2048]
  ssq_c = sum_d y[t,d]^2 over this half's 512 dims [1, 2048]
Host combines: out[b] = rsqrt((ssq0+ssq1)/1024 + eps)[:,None] * (pT0+pT1).T
(The rmsnorm scalar commutes past the Wo matmul.)

Schedule (engine-balanced, ~352us/core on the CoreSim cost model, 2.8x over
the first working version):
 - x arrives host-transposed, so x^T is a straight DMA (quarter-granular
   so the first projection starts early); projections then stream the PE at
   100% (full P-state).  ALL log-domain work (l2-norm rsqrt + log-sigmoid)
   batches into two Ln activations to avoid act-table reloads (was 65).
   k stays RAW in SBUF: ln|k| folds algebraically into the shared exp bias
   (rows), v1 (columns) and nbgp (ks0 term), so k never needs normalizing.
 - scan: chunk-outer over parity-head PAIRS (even head on partitions 0-63,
   odd on 64-127) so 8 independent recurrences pipeline and the pair shares
   one erow/qg/rv/r/z/yt/s0 op where per-head scalars allow.  Per head the
   e1s|e2s tiles fuse into one [128,256] Exp shifted by -55 so off-mask
   entries stay finite in bf16; a 0/+-e^55 mask constant unshifts and masks
   both heads in one Pool TT.  The log-value row-broadcasts run as hi+lo
   bf16 matmul pairs accumulating the f32 value in PSUM (4x cheaper than
   f32 matmuls).  Neumann solve uses 2 applies (error-neutral vs 4), each
   a bf16 (-W)@Z matmul plus one paired DVE add; the state's bf16 shadow
   copy runs on Pool to keep the cross-chunk chain off DVE.
 - phase E is interleaved per 2-chunk group (gate Silu, zt, ssq, Wo matmul,
   pt DMA SP-issued) to fill scan bubbles and keep the output drain short.
PSUM tags pack multiple per-iteration tiles into single banks (bank-granular
allocator) so rotation depth 2 fits in 8 banks.
"""

import numpy as np
import ml_dtypes
from contextlib import ExitStack

import concourse.bass as bass
import concourse.bacc as bacc_mod
import concourse.tile as tile
from concourse import mybir
from concourse.bass_utils import run_bass_kernel_spmd
from concourse.masks import make_identity

F32 = mybir.dt.float32
BF16 = mybir.dt.bfloat16
AF = mybir.ActivationFunctionType
ALU = mybir.AluOpType

B, S, D = 4, 2048, 1024
H, HD = 16, 64          # global heads
HL = 8                  # heads per core
DL = HL * HD            # 512 dims per core
C = 128                 # chunk length
NCH = S // C            # 16 chunks
NT = S // 128           # 16 time tiles (== chunks)
KD = D // 128           # 8 contraction tiles
NEUMANN = 3             # series terms (4 applies)
BIG = 1e30
SHIFT = 55.0            # exponent shift keeping masked exps finite
E55 = float(np.exp(55.0))

_cache = {}


def _build():
    nc = bacc_mod.Bacc()
    xt_d = nc.dram_tensor("xt", [D, S], BF16, kind="ExternalInput")
    wq_d = nc.dram_tensor("wq", [D, DL], BF16, kind="ExternalInput")
    wk_d = nc.dram_tensor("wk", [D, DL], BF16, kind="ExternalInput")
    wv_d = nc.dram_tensor("wv", [D, DL], BF16, kind="ExternalInput")
    wab_d = nc.dram_tensor("wab", [D, 2 * HL], BF16, kind="ExternalInput")
    wg_d = nc.dram_tensor("wg", [D, DL], BF16, kind="ExternalInput")
    wo_d = nc.dram_tensor("wo", [DL, D], BF16, kind="ExternalInput")
    gn_d = nc.dram_tensor("gn", [DL], F32, kind="ExternalInput")
    pt_d = nc.dram_tensor("pt", [D, S], F32, kind="ExternalOutput")
    ssq_d = nc.dram_tensor("ssq", [1, S], F32, kind="ExternalOutput")

    with ExitStack() as ctx:
        tc = ctx.enter_context(tile.TileContext(nc))
        const = ctx.enter_context(tc.tile_pool(name="const", bufs=1))
        persist = ctx.enter_context(tc.tile_pool(name="persist", bufs=1))

        # ---- constants ----
        ident = const.tile([128, 128], F32)
        make_identity(nc, ident)
        identb = const.tile([128, 128], BF16)
        nc.vector.tensor_copy(identb, ident)
        # LT[p, m] = 1 iff p <= m  (lhsT for inclusive cumsum along positions)
        lt = const.tile([128, 128], F32)
        nc.vector.memset(lt, 1.0)
        nc.gpsimd.affine_select(out=lt, in_=lt, compare_op=ALU.is_ge,
                                fill=0.0, base=0, pattern=[[1, 128]],
                                channel_multiplier=-1)
        # 0/1 masks (bf16) in [sigma(part), t(free)]; strict half pre-negated
        # so wt comes out negated for the Neumann add.
        m01 = const.tile([128, 4, 128], BF16)   # [-strict|incl] x 2 heads
        for hh in (0, 2):
            nc.vector.memset(m01[:, hh, :], -E55)    # sigma < t -> -E55 else 0
            nc.gpsimd.affine_select(out=m01[:, hh, :], in_=m01[:, hh, :],
                                    compare_op=ALU.is_ge, fill=0.0, base=-1,
                                    pattern=[[1, 128]], channel_multiplier=-1)
            nc.vector.memset(m01[:, hh + 1, :], E55)  # sigma <= t -> +E55 else 0
            nc.gpsimd.affine_select(out=m01[:, hh + 1, :], in_=m01[:, hh + 1, :],
                                    compare_op=ALU.is_ge, fill=0.0, base=0,
                                    pattern=[[1, 128]], channel_multiplier=-1)
        ones_col = const.tile([128, 1], BF16)
        nc.vector.memset(ones_col, 1.0)
        gn_sb = const.tile([128, 4], F32)  # g_norm half, col j = dims j*128..
        nc.gpsimd.dma_start(out=gn_sb, in_=gn_d.rearrange("(j p) -> p j", p=128))

        # ---- persistent activations ----
        xtb = persist.tile([128, KD, S], BF16)       # x^T  [d, t]
        knat = persist.tile([128, NT, DL], BF16)      # k (l2-normed) [t, (l e)]
        vnat = persist.tile([128, NT, DL], BF16)     # v [t, (l e)]
        qnat = persist.tile([128, NT, DL], BF16)     # q (l2-normed) [t, (l e)]
        qt = persist.tile([128, 4, S], BF16)         # q^T [(l e), t] (4 row-tiles)
        kt = persist.tile([128, 4, S], BF16)
        yt = persist.tile([128, 4, S], BF16)         # y^T [(l e), t]
        la_src = persist.tile([128, 128], F32)       # log alpha  [pos, (c l)]
        lb_src = persist.tile([128, 128], F32)       # log beta
        beta_a = persist.tile([128, 128], F32)       # beta
        lg_a = persist.tile([128, 128], F32)         # cumsum log alpha (incl)
        nlg_sh = persist.tile([128, 128], F32)       # -lg_a - SHIFT
        elgp = persist.tile([128, 128], F32)         # exp(lg_a + SHIFT)
        v1_b = persist.tile([128, 128], F32)         # (lg_ex + log beta)^T
        lg_b = persist.tile([128, 128], F32)         # lg_a^T
        v1hi = persist.tile([128, 128], BF16)        # bf16 hi/lo split of v1_b
        v1lo = persist.tile([128, 128], BF16)
        lghi = persist.tile([128, 128], BF16)        # bf16 hi/lo split of lg_b
        lglo = persist.tile([128, 128], BF16)
        nbgp = persist.tile([128, 128], F32)         # -beta*exp(lg_ex)

        # =========== phase B: projections q,k,v,ab + x transpose ===========
        with tc.tile_pool(name="wpool", bufs=1) as wpool, \
             tc.tile_pool(name="ppool", bufs=4) as ppool, \
             tc.tile_pool(name="pj_ps", bufs=2, space="PSUM") as pj_ps:
            wq_sb = wpool.tile([128, KD, DL], BF16, tag="wq")
            wk_sb = wpool.tile([128, KD, DL], BF16, tag="wk")
            wv_sb = wpool.tile([128, KD, DL], BF16, tag="wv")
            wab_sb = wpool.tile([128, KD, 2 * HL], BF16, tag="wab")
            nsq = wpool.tile([128, NT, 2 * HL], F32, tag="nsq")  # |q|^2, |k|^2
            en_all = wpool.tile([128, NT, 2 * HL], F32, tag="en")  # exp(-z_ab)
            rn_all = wpool.tile([128, NT, 2 * HL], F32, tag="rn")
            for w_sb, w_d in ((wq_sb, wq_d), (wk_sb, wk_d), (wv_sb, wv_d)):
                nc.gpsimd.dma_start(out=w_sb, in_=w_d.rearrange("(k p) n -> p k n", p=128))
            nc.gpsimd.dma_start(out=wab_sb, in_=wab_d.rearrange("(k p) n -> p k n", p=128))

            # pass 1: projections, raw q/k stash, norms, exp(-z).
            # x arrives host-transposed; xtb is a straight DMA.
            xtv = xt_d.rearrange("(k p) t -> p k t", p=128)
            for qtr in range(4):
                hs = slice(qtr * (S // 4), (qtr + 1) * (S // 4))
                for d in range(KD):
                    nc.sync.dma_start(out=xtb[:, d, hs], in_=xtv[:, d, hs])
            def v_proj(m):
                ps_v = pj_ps.tile([128, DL], F32, tag="psv", bufs=2,
                                  name=f"psv{m}")
                for d in range(KD):
                    lw = xtb[:, d, m * 128:(m + 1) * 128]
                    nc.tensor.matmul(ps_v, lw, wv_sb[:, d, :],
                                     start=(d == 0), stop=(d == KD - 1))
                nc.scalar.activation(vnat[:, m, :], ps_v, AF.Copy)

            for m in range(NT):
                # projections for this time tile (v for the second half is
                # deferred past the phase-C boundary to fill the scan ramp)
                ps_q = pj_ps.tile([128, DL], F32, tag="psq", bufs=2)
                ps_k = pj_ps.tile([128, DL], F32, tag="psk", bufs=2)
                ps_ab = pj_ps.tile([128, 2 * HL], F32, tag="psab", bufs=1)
                for d in range(KD):
                    lw = xtb[:, d, m * 128:(m + 1) * 128]
                    st, sp = d == 0, d == KD - 1
                    nc.tensor.matmul(ps_q, lw, wq_sb[:, d, :], start=st, stop=sp)
                    nc.tensor.matmul(ps_k, lw, wk_sb[:, d, :], start=st, stop=sp)
                    nc.tensor.matmul(ps_ab, lw, wab_sb[:, d, :], start=st, stop=sp)
                if m < NT // 2:
                    v_proj(m)
                nc.scalar.activation(qnat[:, m, :], ps_q, AF.Copy)
                nc.scalar.activation(knat[:, m, :], ps_k, AF.Copy)
                nc.scalar.activation(en_all[:, m, :], ps_ab, AF.Exp, scale=-1.0)
                for i, src in enumerate((qnat, knat)):
                    sqb = ppool.tile([128, DL], BF16, tag=f"sq{i}")
                    nc.vector.tensor_tensor(sqb, src[:, m, :], src[:, m, :],
                                            op=ALU.mult)
                    nc.vector.tensor_reduce(
                        nsq[:, m, i * HL:(i + 1) * HL],
                        sqb.rearrange("p (l e) -> p l e", e=HD),
                        axis=mybir.AxisListType.X, op=ALU.add)

            # pass 2+C: log pipeline emitted per sequence-half so half 0
            # completes while phase B still streams the second half
            nlt = wpool.tile([128, NT, 2 * HL], F32, tag="nlt")
            spt = wpool.tile([128, NT, 2 * HL], F32, tag="spt")
            sp1 = ppool.tile([128, NT, 2 * HL], F32, tag="sp1", bufs=1)
            v3 = lambda t: t.rearrange("p (c l) -> p c l", l=HL)

            def logc_half(hh):
                ms = slice(hh * NT // 2, (hh + 1) * NT // 2)
                cols = slice(hh * 64, (hh + 1) * 64)
                rows = slice(hh * 64, (hh + 1) * 64)
                nc.vector.tensor_scalar_add(sp1[:, ms, :], en_all[:, ms, :],
                                            1.0)
                nc.scalar.activation(nlt[:, ms, :], nsq[:, ms, :], AF.Ln)
                nc.scalar.activation(spt[:, ms, :], sp1[:, ms, :], AF.Ln)
                nc.scalar.activation(rn_all[:, ms, :], nlt[:, ms, :], AF.Exp,
                                     scale=-0.5)
                lav = la_src.rearrange("p (c l) -> p c l", l=HL)
                lbv = lb_src.rearrange("p (c l) -> p c l", l=HL)
                nc.vector.tensor_scalar_mul(lav[:, ms, :], spt[:, ms, 0:HL],
                                            -1.0)
                nc.vector.tensor_scalar_mul(lbv[:, ms, :],
                                            spt[:, ms, HL:2 * HL], -1.0)
                nc.scalar.activation(
                    beta_a.rearrange("p (c l) -> p c l", l=HL)[:, ms, :],
                    spt[:, ms, HL:2 * HL], AF.Exp, scale=-1.0)
                # l2-normalize q for this half (k stays raw; ln|k| folds)
                for m in range(hh * NT // 2, (hh + 1) * NT // 2):
                    rnq = rn_all[:, m, 0:HL].unsqueeze(-1) \
                        .broadcast_to([128, HL, HD])
                    qv = qnat[:, m, :].rearrange("p (l e) -> p l e", e=HD)
                    eng = nc.gpsimd if m % 2 == 0 else nc.vector
                    eng.tensor_tensor(qv, qv, rnq, op=ALU.mult)
                # log-gamma pipeline, column-sliced
                ps = pj_ps.tile([128, 128], F32, tag="lgt", bufs=1,
                                name=f"lgps{hh}")
                nc.tensor.matmul(ps[:, 0:64], lt, la_src[:, cols], start=True,
                                 stop=True)
                nc.scalar.activation(lg_a[:, cols], ps[:, 0:64], AF.Copy)
                nc.vector.tensor_scalar(nlg_sh[:, cols], lg_a[:, cols], -1.0,
                                        -SHIFT, op0=ALU.mult, op1=ALU.add)
                nltk_v = nlt[:, ms, HL:2 * HL]
                rnk_v = rn_all[:, ms, HL:2 * HL]
                nc.vector.scalar_tensor_tensor(v3(nlg_sh)[:, ms, :], nltk_v,
                                               -0.5, v3(nlg_sh)[:, ms, :],
                                               op0=ALU.mult, op1=ALU.add)
                nc.scalar.activation(elgp[:, cols], nlg_sh[:, cols], AF.Exp,
                                     scale=-1.0)
                lgex = ppool.tile([128, 64], F32, tag="lgex",
                                  name=f"lgex{hh}")
                nc.vector.tensor_sub(lgex, lg_a[:, cols], la_src[:, cols])
                egex = ppool.tile([128, 64], F32, tag="egex",
                                  name=f"egex{hh}")
                nc.scalar.activation(egex, lgex, AF.Exp)
                nc.vector.scalar_tensor_tensor(nbgp[:, cols], egex, -1.0,
                                               beta_a[:, cols],
                                               op0=ALU.mult, op1=ALU.mult)
                nc.vector.tensor_tensor(v3(nbgp)[:, ms, :], v3(nbgp)[:, ms, :],
                                        rnk_v, op=ALU.mult)
                v1a = ppool.tile([128, 64], F32, tag="v1a", name=f"v1a{hh}")
                nc.vector.tensor_add(v1a, lgex, lb_src[:, cols])
                nc.vector.scalar_tensor_tensor(
                    v3(v1a).rearrange("p c l -> p (c l)")
                    .rearrange("p (c l) -> p c l", l=HL), nltk_v, -0.5,
                    v1a.rearrange("p (c l) -> p c l", l=HL),
                    op0=ALU.mult, op1=ALU.add)
                ps2 = pj_ps.tile([128, 128], F32, tag="lgt", bufs=1,
                                 name=f"lgt2{hh}")
                nc.tensor.matmul(ps2[rows, :], v1a, ident, start=True,
                                 stop=True, tile_position=(0, 64 * hh))
                nc.scalar.activation(v1_b[rows, :], ps2[rows, :], AF.Copy)
                ps3 = pj_ps.tile([128, 128], F32, tag="lgt", bufs=1,
                                 name=f"lgt3{hh}")
                nc.tensor.matmul(ps3[rows, :], lg_a[:, cols], ident,
                                 start=True, stop=True,
                                 tile_position=(0, 64 * hh))
                nc.scalar.activation(lg_b[rows, :], ps3[rows, :], AF.Copy)
                for full, hi, lo in ((v1_b, v1hi, v1lo), (lg_b, lghi, lglo)):
                    nc.vector.tensor_copy(hi[rows, :], full[rows, :])
                    nc.vector.tensor_sub(lo[rows, :], full[rows, :],
                                         hi[rows, :])

            logc_half(0)
            logc_half(1)
            for m in range(NT // 2, NT):
                v_proj(m)

        # ====== phase D+E: scan with interleaved transposes + output ======
        with tc.tile_pool(name="spool", bufs=6) as sp, \
             tc.tile_pool(name="state", bufs=1) as statep, \
             tc.tile_pool(name="sc_ps", bufs=2, space="PSUM") as scps:
            wg_sb = statep.tile([128, KD, DL], BF16, tag="wg")
            wo_sb = statep.tile([128, 4, D], BF16, tag="wo")
            nc.gpsimd.dma_start(out=wg_sb, in_=wg_d.rearrange("(k p) n -> p k n", p=128))
            nc.gpsimd.dma_start(out=wo_sb, in_=wo_d.rearrange("(j p) n -> p j n", p=128))
            # states: head parity on partitions (odd heads at base 64)
            s0 = statep.tile([128, HL // 2, HD], F32)
            s0b = statep.tile([128, HL // 2, HD], BF16)
            nc.vector.memset(s0, 0.0)
            nc.vector.memset(s0b, 0.0)
            for c in range(NCH):
                # build q^T/k^T column tiles for this chunk
                for srcb, dst in ((qnat[:, c, :], qt), (knat[:, c, :], kt)):
                    for j in range(4):
                        tps = scps.tile([128, 128], BF16, tag="g", bufs=2)
                        nc.tensor.transpose(tps, srcb[:, j * 128:(j + 1) * 128], identb)
                        dd = dst[:, j, c * 128:(c + 1) * 128]
                        nc.scalar.activation(dd, tps, AF.Copy)
                for jp in range(4):        # parity head pair (2jp, 2jp+1)
                    ccols = slice(c * 128, (c + 1) * 128)
                    kthf = kt[:, jp, ccols]
                    qthf = qt[:, jp, ccols]
                    hd_ = []
                    erow2 = sp.tile([128, 128], F32, tag="erow")
                    g12 = scps.tile([128, 4, 128], F32, tag="g", bufs=2)
                    e12 = sp.tile([128, 4, 128], BF16, tag="e12", bufs=4)
                    me = sp.tile([128, 4, 128], F32, tag="me", bufs=4)
                    kk_ps = scps.tile([128, 4, 128], F32, tag="mm1", bufs=1)
                    for h in (0, 1):
                        u = c * HL + 2 * jp + h
                        iub = bass.AP(tensor=identb.tensor,
                                      offset=identb.offset + u,
                                      ap=[identb.ap[0], [0, 128]])
                        # per head: bcast v1[t] then lg[t] (hi+lo bf16 pairs)
                        gh = g12[:, 2 * h:2 * h + 2, :]
                        nc.tensor.matmul(gh[:, 0, :], iub, v1hi, start=True, stop=False)
                        nc.tensor.matmul(gh[:, 0, :], iub, v1lo, start=False, stop=True)
                        nc.tensor.matmul(gh[:, 1, :], iub, lghi, start=True, stop=False)
                        nc.tensor.matmul(gh[:, 1, :], iub, lglo, start=False, stop=True)
                        nc.scalar.activation(e12[:, 2 * h:2 * h + 2, :], gh, AF.Exp,
                                             bias=nlg_sh[:, u:u + 1])
                    nc.gpsimd.tensor_tensor(me, m01, e12, op=ALU.mult)
                    for h in (0, 1):
                        u = c * HL + 2 * jp + h
                        pb = h * 64
                        psl = slice(pb, pb + 64)
                        kth = kt[psl, jp, ccols]
                        qth = qt[psl, jp, ccols]
                        # erow2[h rows] = exp(lg[t]) for this head (exact unshift)
                        nc.gpsimd.tensor_tensor(
                            erow2[psl, :], e12[psl, 2 * h + 1, :],
                            elgp[psl, u:u + 1].broadcast_to([64, 128]), op=ALU.mult)
                        akk = kk_ps[:, 2 * h, :]
                        aqk_ps = kk_ps[:, 2 * h + 1, :]
                        nc.tensor.matmul(akk, kth, kth, start=True, stop=True)
                        wt = sp.tile([128, 128], BF16, tag="wt", bufs=8)
                        nc.vector.tensor_tensor(wt, me[:, 2 * h, :], akk, op=ALU.mult)
                        nc.tensor.matmul(aqk_ps, kth, qth, start=True, stop=True)
                        aqk = sp.tile([128, 128], BF16, tag="aqk", bufs=8)
                        nc.vector.tensor_tensor(aqk, aqk_ps, me[:, 2 * h + 1, :],
                                                op=ALU.mult)
                        hd_.append((u, pb, psl, kth, qth, me, wt, aqk))
                    # paired: Q^T * gamma_t via the stitched erow2
                    qg = sp.tile([128, 128], BF16, tag="qg")
                    nc.gpsimd.tensor_tensor(qg, qthf, erow2, op=ALU.mult)
                    # paired RHS: R = beta*V - (beta*gamma_ex) .* (K @ S0)
                    u0 = c * HL + 2 * jp
                    rv = sp.tile([128, 2, HD], F32, tag="rv")
                    nc.gpsimd.tensor_tensor(
                        rv, vnat[:, c, 2 * jp * HD:(2 * jp + 2) * HD]
                        .rearrange("p (h e) -> p h e", e=HD),
                        beta_a[:, u0:u0 + 2].unsqueeze(-1)
                        .broadcast_to([128, 2, HD]), op=ALU.mult)
                    zbank = scps.tile([128, 8, HD], F32, tag="mm2", bufs=2)
                    osb_ps = scps.tile([128, 5, HD], F32, tag="mm3", bufs=1)
                    r = sp.tile([128, 2, HD], BF16, tag="r")
                    for h, (u, pb, psl, kth, qth, me, wt, aqk) in enumerate(hd_):
                        ks0 = zbank[:, h, :]
                        nc.tensor.matmul(ks0, kth, s0b[psl, jp, :], start=True,
                                         stop=True)
                        nc.vector.scalar_tensor_tensor(r[:, h, :], ks0,
                                                       nbgp[:, u:u + 1], rv[:, h, :],
                                                       op0=ALU.mult, op1=ALU.add)
                    # truncated Neumann: Z <- R + (-W) Z, both heads per step
                    z = r
                    for it in range(NEUMANN - 1):
                        zp = zbank[:, 2 + 2 * it:4 + 2 * it, :]
                        for h, (u, pb, psl, kth, qth, me, wt, aqk) in enumerate(hd_):
                            nc.tensor.matmul(zp[:, h, :], wt, z[:, h, :],
                                             start=True, stop=True)
                        z2 = sp.tile([128, 2, HD], BF16, tag=f"z{it % 2}")
                        nc.vector.tensor_add(z2, r, zp)
                        z = z2
                    # O^T = U^T AqkT + S0^T (gamma Q^T), both heads in one tile
                    ot = osb_ps[:, 0:2, :].rearrange("p a b -> p (a b)")
                    for h, (u, pb, psl, kth, qth, me, wt, aqk) in enumerate(hd_):
                        nc.tensor.matmul(ot[psl, :], z[:, h, :], aqk, start=True,
                                         stop=False, tile_position=(0, pb))
                        nc.tensor.matmul(ot[psl, :], s0b[psl, jp, :], qg[psl, :],
                                         start=False, stop=True,
                                         tile_position=(pb, pb))
                    ytd = yt[:, jp, ccols]
                    nc.scalar.activation(ytd, ot, AF.Copy)
                    # state update; ubar unshifts e2s[127] by E55 (in the mask)
                    snew = osb_ps[:, 2, :]
                    for h, (u, pb, psl, kth, qth, me, wt, aqk) in enumerate(hd_):
                        ubar = sp.tile([128, HD], BF16, tag=f"ub{h}")
                        nc.vector.tensor_scalar_mul(ubar, z[:, h, :],
                                                    me[:, 2 * h + 1, 127:128])
                        nc.tensor.matmul(snew[psl, :],
                                         knat[:, c, (2 * jp + h) * HD:
                                              (2 * jp + h + 1) * HD],
                                         ubar, start=True, stop=True,
                                         tile_position=(0, pb))
                    nc.vector.scalar_tensor_tensor(s0[:, jp, :], s0[:, jp, :],
                                                   erow2[:, 127:128], snew,
                                                   op0=ALU.mult, op1=ALU.add)
                    nc.gpsimd.tensor_copy(s0b[:, jp, :], s0[:, jp, :])
                egroups = []
                if c % 2 == 1:
                    egroups = [((c - 1) * 128, (c + 1) * 128)]
                for (c0, c1) in egroups:
                    cw = c1 - c0
                    cols = slice(c0, c1)
                    ztg = sp.tile([128, 4, 512], BF16, tag="ztg", bufs=2)
                    sqys = []
                    for j in range(4):
                        gps = scps.tile([128, 512], F32, tag="emm", bufs=2)
                        for d in range(KD):
                            nc.tensor.matmul(gps[:, :cw], wg_sb[:, d, j * 128:(j + 1) * 128],
                                             xtb[:, d, cols], start=(d == 0),
                                             stop=(d == KD - 1))
                        gt = sp.tile([128, 512], BF16, tag="gt", bufs=3)
                        nc.scalar.activation(gt[:, :cw], gps[:, :cw], AF.Silu)
                        nc.vector.scalar_tensor_tensor(ztg[:, j, :cw], yt[:, j, cols],
                                                       gn_sb[:, j:j + 1], gt[:, :cw],
                                                       op0=ALU.mult, op1=ALU.mult)
                        sqy = sp.tile([128, 512], BF16, tag=f"sqy{j}", bufs=1)
                        nc.vector.tensor_tensor(sqy[:, :cw], yt[:, j, cols],
                                                yt[:, j, cols], op=ALU.mult)
                        sqys.append(sqy)
                    spt_ = scps.tile([128, 512], F32, tag="emm", bufs=2)
                    sps = spt_[0:1, :cw]
                    for j in range(4):
                        nc.tensor.matmul(sps, ones_col, sqys[j][:, :cw],
                                         start=(j == 0), stop=(j == 3))
                    ssq_g = sp.tile([1, 512], F32, tag="ssqg", bufs=2)
                    nc.vector.tensor_copy(ssq_g[:, :cw], sps)
                    nc.sync.dma_start(out=ssq_d[:, cols], in_=ssq_g[:, :cw])
                    for mo in range(8):
                        ops_ = scps.tile([128, 512], F32, tag="emm", bufs=2)
                        for j in range(4):
                            nc.tensor.matmul(ops_[:, :cw],
                                             wo_sb[:, j, mo * 128:(mo + 1) * 128],
                                             ztg[:, j, :cw], start=(j == 0),
                                             stop=(j == 3))
                        osb = sp.tile([128, 512], F32, tag="osb", bufs=2)
                        if mo % 2 == 0:
                            nc.scalar.activation(osb[:, :cw], ops_[:, :cw], AF.Copy)
                        else:
                            nc.vector.tensor_copy(osb[:, :cw], ops_[:, :cw])
                        nc.sync.dma_start(out=pt_d[mo * 128:(mo + 1) * 128, cols],
                                          in_=osb[:, :cw])
        import bass_rust as _bass_rust
        from concourse.hw_specs import get_activation_tables

        def _two_table_loads():
            keep = ("natural_log_exp_and_others", "silu_and_others")
            tables = [(k, (v if k in keep else set()))
                      for k, v in get_activation_tables(nc.m.arch).items()]
            _bass_rust.insert_act_table_loads(nc, tables)

        nc.insert_act_table_loads = _two_table_loads
    nc.compile()
    return nc


def kernel(**inputs):
    x = np.ascontiguousarray(np.asarray(inputs["x"], dtype=np.float32))
    Wq = np.asarray(inputs["Wq"], dtype=np.float32)
    Wk = np.asarray(inputs["Wk"], dtype=np.float32)
    Wv = np.asarray(inputs["Wv"], dtype=np.float32)
    Wa = np.asarray(inputs["Wa"], dtype=np.float32)
    Wb = np.asarray(inputs["Wb"], dtype=np.float32)
    Wg = np.asarray(inputs["Wg"], dtype=np.float32)
    Wo = np.asarray(inputs["Wo"], dtype=np.float32)
    gn = np.asarray(inputs["g_norm"], dtype=np.float32)

    if "nc" not in _cache:
        _cache["nc"] = _build()
    nc = _cache["nc"]

    bf = ml_dtypes.bfloat16
    in_maps = []
    for core in range(8):
        b, hh = core // 2, core % 2
        cs, ch = slice(hh * DL, (hh + 1) * DL), slice(hh * HL, (hh + 1) * HL)
        in_maps.append({
            "xt": np.ascontiguousarray(x[b].T.astype(bf)),
            "wq": np.ascontiguousarray(Wq[:, cs].astype(bf)),
            "wk": np.ascontiguousarray(Wk[:, cs].astype(bf)),
            "wv": np.ascontiguousarray(Wv[:, cs].astype(bf)),
            "wab": np.ascontiguousarray(
                np.concatenate([Wa[:, ch], Wb[:, ch]], axis=1).astype(bf)),
            "wg": np.ascontiguousarray(Wg[:, cs].astype(bf)),
            "wo": np.ascontiguousarray(Wo[cs, :].astype(bf)),
            "gn": np.ascontiguousarray(gn[cs]),
        })
    res = run_bass_kernel_spmd(nc, in_maps, core_ids=list(range(8)))
    _cache["last_result"] = res
    out = np.zeros((B, S, D), np.float32)
    for b in range(B):
        r0, r1 = res.results[2 * b], res.results[2 * b + 1]
        p = (r0["pt"] + r1["pt"]).T
        ssq = (r0["ssq"] + r1["ssq"]).reshape(S, 1)
        inv_rms = 1.0 / np.sqrt(ssq / D + 1e-5)
        out[b] = p * inv_rms
    return out

